# revision 13
# baseline (speedup 1.0000x reference)
"""ContentGuidedAttention Trainium2 kernel.

Full NxN single-head cross-attention + out-proj + residual + LayerNorm,
for B=4, C=256, H=W=64 (N=4096 tokens), distributed over 8 NeuronCores:
core i handles batch i//2, query-half i%2 (2048 queries, all 4096 keys).
No collectives: K/V are computed redundantly on the two cores sharing a
batch (~5% extra FLOPs).

Layout strategy (channel-major, zero transposes, fp8 DoubleRow on every
matmul the PE streams):
  - Q^T/K^T as [C, n] fp8e4 (q/k weights prescaled by 16, compensated in
    the softmax exp scale); V token-major [n, C] fp8e4 via DR matmuls,
    evacuated on GpSimd (DVE is the preamble bottleneck)
  - S^T = K Q^T fp8 DR; exp on ACT -> P^T fp8e4
  - softmax denominator: DR ones-vector matmuls accumulate into a [1, q]
    psum row; 1/denom via a single DVE reciprocal_approx_fast (keeps the
    ACT queue exp-only), then gpsimd partition_broadcast
  - PV: O^T[c, q] = sum_k V[k,c] P^T[k,q], fp8 DR; O^T evacuated to fp8
    so the out-proj is DR too; residual uses a separate f32 copy of low
  - scheduling: the steady-state loop WEAVES 2-si S-matmul pairs between
    ~1.5us chunks of PV/outproj/LN/denominator matmuls so the in-order
    PE queue never parks behind an S matmul whose psum buffer is still
    being drained by ACT (st_ps has only 2 bufs), and ACT always has exp
    work queued.  Same fine-grained interleave in the projection
    preamble.  ~35 dummy 128-col matmuls at t~3.5us warm the PE HAM
    clock-gate before real work; inputs ride 4 DMA queues in parallel.
  - tail: last block's out-proj + LN run as two 256-query halves so the
    serial ACT/DVE chain of one half overlaps the other's matmuls.
"""

import ml_dtypes
import numpy as np

import concourse.bass as bass
import concourse.mybir as mybir
import concourse.tile as tile
from concourse import bacc
from concourse.bass import ds, ts
from concourse.bass_utils import run_bass_kernel_spmd

F32 = mybir.dt.float32
F32R = mybir.dt.float32r
BF16 = mybir.dt.bfloat16
F8 = mybir.dt.float8e4
AF = mybir.ActivationFunctionType
OP = mybir.AluOpType
DR = mybir.MatmulPerfMode.DoubleRow

B = 4
C = 256
N = 4096          # tokens per batch
NQ = 2048         # queries per core
QB = 512          # query block
NQB = NQ // QB    # 4
NKC = N // 128    # 32 key chunks
NKR = 4           # key ranges (1024 keys each) for K^T / V tiles
QK_PRE = 16.0     # host-side prescale on q/k weights (fp8 range centering)
SCALE = (C // 8) ** -0.5
EXP_SCALE = SCALE / (QK_PRE * QK_PRE)
LN_EPS = 1e-5

_CACHE = {}


def _build_nc():
    nc = bacc.Bacc("TRN2", target_bir_lowering=False, debug=False)

    low_d = nc.declare_dram_parameter("low", [C, NQ], F32R, isOutput=False)
    lowf8_d = nc.declare_dram_parameter("lowf8", [C, NQ], F8, isOutput=False)
    high_d = nc.declare_dram_parameter("high", [C, N], F8, isOutput=False)
    # weights are passed pre-transposed: [c_in, c_out], fp8
    wq_d = nc.declare_dram_parameter("wq", [C, C], F8, isOutput=False)
    wk_d = nc.declare_dram_parameter("wk", [C, C], F8, isOutput=False)
    wv_d = nc.declare_dram_parameter("wv", [C, C], F8, isOutput=False)
    wo_d = nc.declare_dram_parameter("wo", [C, C], F8, isOutput=False)
    # qb, kb, ob, lng, lnb prepacked host-side as [128, 10]
    pvec_d = nc.declare_dram_parameter("pvec", [128, 10], F32, isOutput=False)
    out_d = nc.declare_dram_parameter("out", [C, NQ], F32, isOutput=True)

    with tile.TileContext(nc) as tc:
        with (
            tc.tile_pool(name="persist", bufs=1) as pp,
            tc.tile_pool(name="high", bufs=4) as high_pool,
            tc.tile_pool(name="pt", bufs=8) as pt_pool,
            tc.tile_pool(name="ot", bufs=3) as ot_pool,
            tc.tile_pool(name="scratch", bufs=3) as scr_pool,
            tc.tile_pool(name="rowscr", bufs=1) as row_pool,
            tc.tile_pool(name="outsb", bufs=4) as out_pool,
            tc.tile_pool(name="st_ps", bufs=2, space="PSUM") as st_ps,
            tc.tile_pool(name="acc_ps", bufs=3, space="PSUM") as acc_ps,
            tc.tile_pool(name="row_ps", bufs=1, space="PSUM") as row_ps,
        ):
            # ---------------- constants + PE warm-up ----------------
            stage = pp.tile([128, 128], F32)
            nc.vector.memset(stage[:, :], 1.0)
            ones128 = pp.tile([128, 1], F32R)    # partition-reduce lhsT
            nc.vector.tensor_copy(ones128[:, :], stage[:, 0:1])
            # tiny exp: pulls the ACT table load to ~3.6us (ACT idle)
            tinyrow = pp.tile([1, 1], F32)
            nc.scalar.activation(out=tinyrow[:, :], in_=stage[0:1, 0:1],
                                 func=AF.Exp)
            # ~35 dummy matmuls release the PE HAM clock-gate (~3.4us of
            # sustained activity) before the first real matmul at ~8us
            warm_ps = row_ps.tile([1, 128], F32, tag="row")
            for w in range(35):
                nc.tensor.matmul(
                    out=warm_ps[:, :], lhsT=ones128[:, :],
                    rhs=stage[:, :].bitcast(F32R),
                    start=True, stop=True, skip_group_check=True,
                )
            ones2f8 = pp.tile([128, 2, 16], F8)  # DoubleRow denom lhsT
            nc.vector.tensor_copy(ones2f8[:, :, 0], stage[:, 0:2])
            ones_col = pp.tile([1, 128], F32R)   # K=1 row-broadcast lhsT
            nc.vector.tensor_copy(ones_col[:, :], stage[0:1, :])
            epsb = pp.tile([1, 1], F32)          # LN epsilon bias
            nc.vector.memset(epsb[:, :], LN_EPS)

            # ---------------- input DMAs on 4 parallel queues --------
            wk_sb = pp.tile([128, 2, C], F8)
            wv_sb = pp.tile([128, 2, C], F8)
            wq_sb = pp.tile([128, 2, C], F8)
            wo_sb = pp.tile([128, 2, C], F8)
            pvec = pp.tile([128, 10], F32)
            lowf8_sb = pp.tile([128, 2, NQ], F8)
            low_sb = pp.tile([128, 2, NQ], F32R)
            for j in range(2):
                nc.scalar.dma_start(out=wk_sb[:, j, :], in_=wk_d[ds(j * 128, 128), :])
            nc.gpsimd.dma_start(out=pvec[:, :], in_=pvec_d[:, :])
            for j in range(2):
                nc.gpsimd.dma_start(out=wq_sb[:, j, :], in_=wq_d[ds(j * 128, 128), :])
            for j in range(2):
                nc.gpsimd.dma_start(out=wv_sb[:, j, :], in_=wv_d[ds(j * 128, 128), :])
            for j in range(2):
                nc.gpsimd.dma_start(out=wo_sb[:, j, :], in_=wo_d[ds(j * 128, 128), :])
            hi_tiles = [
                high_pool.tile([128, 2, 1024], F8, name=f"hi{r}")
                for r in range(NKR)
            ]
            # range 0 rides first, split in h-halves so the very first
            # K-projection matmul waits on a [128,512] transfer only
            for h in range(2):
                for j in range(2):
                    nc.sync.dma_start(
                        out=hi_tiles[0][:, j, ds(h * 512, 512)],
                        in_=high_d[ds(j * 128, 128), ds(h * 512, 512)],
                    )
            for j in range(2):
                nc.sync.dma_start(out=lowf8_sb[:, j, :], in_=lowf8_d[ds(j * 128, 128), :])
            for r in range(1, NKR):
                for j in range(2):
                    nc.sync.dma_start(
                        out=hi_tiles[r][:, j, :],
                        in_=high_d[ds(j * 128, 128), ds(r * 1024, 1024)],
                    )
            for j in range(2):
                nc.sync.dma_start(out=low_sb[:, j, :], in_=low_d[ds(j * 128, 128), :])

            QBIAS, KBIAS, OBIAS, LNG, LNB = 0, 2, 4, 6, 8

            kt_sb = [
                pp.tile([128, 2, 1024], F8, name=f"kt{r}", tag=f"kt{r}")
                for r in range(NKR)
            ]
            v_sb = [
                pp.tile([128, 8, C], F8, name=f"v{r}", tag=f"v{r}")
                for r in range(NKR)
            ]
            qt_all = pp.tile([128, 2, NQ], F8)

            # ---------------- work units ----------------
            def k_unit(r, h, split_j=False):
                # K^T: out [cout, k] = sum_cin wk[cin, cout] high[cin, k]
                for c in range(2):
                    kps = st_ps.tile([128, 512], F32, tag="st")
                    if split_j:
                        # first matmuls only need the first DMA chunks
                        for j in range(2):
                            nc.tensor.matmul(
                                out=kps[:, :],
                                lhsT=wk_sb[:, j, ds(c * 128, 128)],
                                rhs=hi_tiles[r][:, j, ds(h * 512, 512)],
                                start=(j == 0), stop=(j == 1),
                            )
                    else:
                        nc.tensor.matmul(
                            out=kps[:, :],
                            lhsT=wk_sb[:, :, ds(c * 128, 128)],
                            rhs=hi_tiles[r][:, :, ds(h * 512, 512)],
                            start=True, stop=True,
                            perf_mode=DR,
                        )
                    # K bias dropped: a k-independent logit shift per query,
                    # exactly cancelled by softmax
                    nc.vector.tensor_copy(
                        kt_sb[r][:, c, ds(h * 512, 512)], kps[:, :]
                    )

            def v_unit(r, up):
                # V: out [k, cout] = sum_cin high[cin, k] wv[cin, cout]
                # DR over the cin halves; last range evacuates on ACT to
                # balance the preamble DVE load
                vps = st_ps.tile([128, 2, C], F32, tag="st")
                for i in range(2):
                    u = up * 2 + i
                    nc.tensor.matmul(
                        out=vps[:, i, :],
                        lhsT=hi_tiles[r][:, :, ds(u * 128, 128)],
                        rhs=wv_sb[:, :, :],
                        start=True, stop=True,
                        perf_mode=DR,
                    )
                if r >= 2:
                    nc.scalar.activation(
                        out=v_sb[r][:, ds(up * 2, 2), :], in_=vps[:, :, :],
                        func=AF.Copy,
                    )
                else:
                    nc.vector.tensor_copy(
                        v_sb[r][:, ds(up * 2, 2), :], vps[:, :, :]
                    )

            def q_proj(qb4):
                for c in range(2):
                    qps = st_ps.tile([128, QB], F32, tag="st")
                    nc.tensor.matmul(
                        out=qps[:, :],
                        lhsT=wq_sb[:, :, ds(c * 128, 128)],
                        rhs=lowf8_sb[:, :, ds(qb4 * QB, QB)],
                        start=True, stop=True,
                        perf_mode=DR,
                    )
                    nc.vector.tensor_scalar_add(
                        out=qt_all[:, c, ds(qb4 * QB, QB)], in0=qps[:, :],
                        scalar1=pvec[:, ds(QBIAS + c, 1)],
                    )

            def alloc_quarters(b):
                return [
                    pt_pool.tile([128, 8, QB], F8, tag="ptq", name=f"ptq{g}")
                    for g in range(4)
                ]

            quarters = {}

            def s_single(b, si):
                # 1 si = 2 key chunks: 2 S matmuls + 1 exp
                qsl = ds(b * QB, QB)
                sps = st_ps.tile([128, 2, QB], F32, tag="st")
                for u in range(2):
                    kc = si * 2 + u
                    nc.tensor.matmul(
                        out=sps[:, u, :],
                        lhsT=kt_sb[kc // 8][:, :, ds((kc % 8) * 128, 128)],
                        rhs=qt_all[:, :, qsl],
                        start=True, stop=True,
                        perf_mode=DR,
                    )
                nc.scalar.activation(
                    out=quarters[b][si // 4][:, ds((si % 4) * 2, 2), :],
                    in_=sps[:, :, :],
                    func=AF.Exp,
                    scale=EXP_SCALE,
                )

            def s_pair(b, p):
                s_single(b, 2 * p)
                s_single(b, 2 * p + 1)

            def denom_part(b, t0, t1, dps=None):
                # split accumulation: t12-15 can be emitted after other PE
                # work so the last exps of block b have time to land
                if dps is None:
                    dps = row_ps.tile([1, QB], F32, tag="row")
                for t in range(t0, t1):
                    nc.tensor.matmul(
                        out=dps[:, :],
                        lhsT=ones2f8[:, :, 0:1],
                        rhs=quarters[b][t // 4][:, ds((t % 4) * 2, 2), :],
                        start=(t == 0), stop=(t == t1 - 1),
                        perf_mode=DR,
                        skip_group_check=True,
                    )
                return dps

            def pv_part(b, c, t0, t1, ops):
                for t in range(t0, t1):
                    nc.tensor.matmul(
                        out=ops[:, :],
                        lhsT=v_sb[t // 4][:, ds((t % 4) * 2, 2),
                                         ds(c * 128, 128)],
                        rhs=quarters[b][t // 4][:, ds((t % 4) * 2, 2), :],
                        start=(t == 0), stop=(t == t1 - 1),
                        perf_mode=DR,
                        skip_group_check=True,
                    )

            def outproj_y(b, ot, rcp_rep, qo=0, ql=QB):
                qsl = ds(b * QB + qo, ql)
                y_sb = ot_pool.tile([128, 2, ql], F32R, tag="y",
                                    name=f"y{b}_{qo}")
                for c in range(2):
                    pps = acc_ps.tile([128, ql], F32, tag="acc")
                    nc.tensor.matmul(
                        out=pps[:, :],
                        lhsT=wo_sb[:, :, ds(c * 128, 128)],
                        rhs=ot[:, :, ds(qo, ql)],
                        start=True, stop=True,
                        perf_mode=DR,
                    )
                    ysc = scr_pool.tile([128, ql], F32, tag="scr")
                    nc.vector.tensor_mul(
                        out=ysc[:, :], in0=pps[:, :], in1=rcp_rep[:, ds(qo, ql)]
                    )
                    nc.vector.scalar_tensor_tensor(
                        out=y_sb[:, c, :],
                        in0=ysc[:, :],
                        scalar=pvec[:, ds(OBIAS + c, 1)],
                        in1=low_sb[:, c, qsl].bitcast(F32),
                        op0=OP.add, op1=OP.add,
                    )
                return y_sb

            def stats_ln_a(b, y_sb):
                sy_ps = row_ps.tile([1, QB], F32, tag="row")
                for c in range(2):
                    nc.tensor.matmul(
                        out=sy_ps[:, :], lhsT=ones128[:, :],
                        rhs=y_sb[:, c, :], start=(c == 0), stop=(c == 1),
                    )
                murow = row_pool.tile([1, QB], F32, tag="murow")
                nc.vector.tensor_scalar_mul(
                    out=murow[:, :], in0=sy_ps[:, :], scalar1=1.0 / C
                )
                mu_rep = scr_pool.tile([128, QB], F32, tag="murep")
                nc.gpsimd.partition_broadcast(mu_rep[:, :], murow[:, :])
                return murow, mu_rep

            def stats_ln_b(b, y_sb, murow):
                sy2_ps = row_ps.tile([1, QB], F32, tag="row")
                for c in range(2):
                    ysq = scr_pool.tile([128, QB], F32R, tag="ysq")
                    nc.vector.tensor_mul(
                        out=ysq[:, :],
                        in0=y_sb[:, c, :].bitcast(F32),
                        in1=y_sb[:, c, :].bitcast(F32),
                    )
                    nc.tensor.matmul(
                        out=sy2_ps[:, :], lhsT=ones128[:, :],
                        rhs=ysq[:, :], start=(c == 0), stop=(c == 1),
                    )
                # C*var = sy2 - C*mu^2 ; rstd = exp(-0.5 ln((C var)/C + eps))
                mu2row = row_pool.tile([1, QB], F32, tag="mu2row")
                nc.vector.tensor_mul(
                    out=mu2row[:, :], in0=murow[:, :], in1=murow[:, :],
                )
                varrow = row_pool.tile([1, QB], F32, tag="varrow")
                nc.vector.scalar_tensor_tensor(
                    out=varrow[:, :], in0=mu2row[:, :], scalar=-float(C),
                    in1=sy2_ps[:, :], op0=OP.mult, op1=OP.add,
                )
                lnv = row_pool.tile([1, QB], F32, tag="lnv")
                nc.scalar.activation(
                    out=lnv[:, :], in_=varrow[:, :], func=AF.Ln,
                    scale=1.0 / C, bias=epsb[:, :],
                )
                rstdrow = row_pool.tile([1, QB], F32, tag="rstdrow")
                nc.scalar.activation(
                    out=rstdrow[:, :], in_=lnv[:, :], func=AF.Exp, scale=-0.5
                )
                rs_rep = scr_pool.tile([128, QB], F32, tag="rsrep")
                nc.gpsimd.partition_broadcast(rs_rep[:, :], rstdrow[:, :])
                return rs_rep

            def stats_ln_c(b, y_sb, mu_rep, rs_rep):
                qsl = ds(b * QB, QB)
                for c in range(2):
                    yn = scr_pool.tile([128, QB], F32, tag="scr")
                    nc.vector.tensor_sub(
                        out=yn[:, :],
                        in0=y_sb[:, c, :].bitcast(F32),
                        in1=mu_rep[:, :],
                    )
                    nc.vector.tensor_mul(
                        out=yn[:, :], in0=yn[:, :], in1=rs_rep[:, :]
                    )
                    osb = out_pool.tile([128, QB], F32)
                    nc.vector.tensor_scalar(
                        out=osb[:, :], in0=yn[:, :],
                        scalar1=pvec[:, ds(LNG + c, 1)],
                        scalar2=pvec[:, ds(LNB + c, 1)],
                        op0=OP.mult, op1=OP.add,
                    )
                    nc.sync.dma_start(
                        out=out_d[ds(c * 128, 128), qsl], in_=osb[:, :]
                    )

            def stats_ln_last(b, ys, nh, ql):
                # span-critical tail, nh halves stage-interleaved so each
                # engine queue alternates halves and the serial chain of
                # one half hides behind the matmuls of the next.  murow on
                # ACT; rstd broadcast via a K=1 PE matmul into psum.
                sy_ps, murow, sy2_ps = {}, {}, {}
                mu2row, varrow, lnv, rstdrow = {}, {}, {}, {}
                mu_rep, rs_ps = {}, {}
                for h in range(nh):
                    sy_ps[h] = st_ps.tile([1, ql], F32, tag="st",
                                          name=f"syp{h}")
                    for c in range(2):
                        nc.tensor.matmul(
                            out=sy_ps[h][:, :], lhsT=ones128[:, :],
                            rhs=ys[h][:, c, :], start=(c == 0), stop=(c == 1),
                        )
                    murow[h] = row_pool.tile([1, ql], F32, tag="murow",
                                             name=f"mur{h}")
                    nc.scalar.activation(
                        out=murow[h][:, :], in_=sy_ps[h][:, :], func=AF.Copy,
                        scale=1.0 / C,
                    )
                    sy2_ps[h] = st_ps.tile([1, ql], F32, tag="st",
                                           name=f"sy2p{h}")
                    for c in range(2):
                        ysq = scr_pool.tile([128, ql], F32R, tag="ysq")
                        nc.vector.tensor_mul(
                            out=ysq[:, :],
                            in0=ys[h][:, c, :].bitcast(F32),
                            in1=ys[h][:, c, :].bitcast(F32),
                        )
                        nc.tensor.matmul(
                            out=sy2_ps[h][:, :], lhsT=ones128[:, :],
                            rhs=ysq[:, :], start=(c == 0), stop=(c == 1),
                        )
                for h in range(nh):
                    mu2row[h] = row_pool.tile([1, ql], F32, tag="mu2row",
                                              name=f"mu2r{h}")
                    nc.vector.tensor_mul(
                        out=mu2row[h][:, :], in0=murow[h][:, :],
                        in1=murow[h][:, :],
                    )
                    varrow[h] = row_pool.tile([1, ql], F32, tag="varrow",
                                              name=f"varr{h}")
                    nc.vector.scalar_tensor_tensor(
                        out=varrow[h][:, :], in0=mu2row[h][:, :],
                        scalar=-float(C),
                        in1=sy2_ps[h][:, :], op0=OP.mult, op1=OP.add,
                    )
                    mu_rep[h] = scr_pool.tile([128, ql], F32, tag="murep",
                                              name=f"murep{h}")
                    nc.gpsimd.partition_broadcast(mu_rep[h][:, :],
                                                  murow[h][:, :])
                for h in range(nh):
                    lnv[h] = row_pool.tile([1, ql], F32, tag="lnv",
                                           name=f"lnv{h}")
                    nc.scalar.activation(
                        out=lnv[h][:, :], in_=varrow[h][:, :], func=AF.Ln,
                        scale=1.0 / C, bias=epsb[:, :],
                    )
                    rstdrow[h] = row_pool.tile([1, ql], F32R, tag="rstdrow",
                                               name=f"rstdr{h}")
                    nc.scalar.activation(
                        out=rstdrow[h][:, :], in_=lnv[h][:, :], func=AF.Exp,
                        scale=-0.5,
                    )
                    rs_ps[h] = acc_ps.tile([128, ql], F32, tag="acc",
                                           name=f"rsps{h}")
                    nc.tensor.matmul(
                        out=rs_ps[h][:, :], lhsT=ones_col[:, :],
                        rhs=rstdrow[h][:, :], start=True, stop=True,
                    )
                for h in range(nh):
                    qsl = ds(b * QB + h * ql, ql)
                    for c in range(2):
                        yn = scr_pool.tile([128, ql], F32, tag="scr")
                        nc.vector.tensor_sub(
                            out=yn[:, :],
                            in0=ys[h][:, c, :].bitcast(F32),
                            in1=mu_rep[h][:, :],
                        )
                        nc.vector.tensor_mul(
                            out=yn[:, :], in0=yn[:, :], in1=rs_ps[h][:, :]
                        )
                        osb = out_pool.tile([128, ql], F32)
                        nc.vector.tensor_scalar(
                            out=osb[:, :], in0=yn[:, :],
                            scalar1=pvec[:, ds(LNG + c, 1)],
                            scalar2=pvec[:, ds(LNB + c, 1)],
                            op0=OP.mult, op1=OP.add,
                        )
                        nc.sync.dma_start(
                            out=out_d[ds(c * 128, 128), qsl], in_=osb[:, :]
                        )

            # ---------------- preamble: projections woven 1:1 with
            # block-0 S singles so the ACT exp stream starts ~10us in and
            # never starves, and no S matmul parks the in-order PE queue
            # (its psum buffer is 2 exps back, covered by a heavy unit)
            quarters[0] = alloc_quarters(0)
            k_unit(0, 0, split_j=True)
            q_proj(0)
            k_unit(0, 1)
            heavies = (
                [(k_unit, 1, 0), (k_unit, 1, 1), (k_unit, 2, 0),
                 (k_unit, 2, 1), (k_unit, 3, 0), (k_unit, 3, 1)]
                + [(v_unit, r, up) for r in range(3) for up in range(4)
                   ][:10]
            )
            for i in range(16):
                fn, a0, a1 = heavies[i]
                fn(a0, a1)
                s_single(0, i)
            v_unit(2, 2)
            q_proj(1)
            v_unit(2, 3)
            q_proj(2)
            v_unit(3, 0)
            q_proj(3)
            v_unit(3, 1)
            v_unit(3, 2)
            v_unit(3, 3)
            dps = {0: denom_part(0, 0, 16)}

            # ---------------- steady state (b = 0..2) ----------------
            # block 3's PV t0-11 is pre-woven into iteration b=2 so the
            # final iteration is just 8 matmuls + the LN tail chain
            pv3 = {}
            for b in range(NQB - 1):
                nb = b + 1
                # 1/denom: single custom-DVE op (ACT stays exp-only)
                rcprow = row_pool.tile([1, QB], F32, tag="rcprow",
                                       name=f"rcprow{b}")
                nc.vector.reciprocal_approx_fast(
                    out=rcprow[:, :], in_=dps[b][:, :]
                )
                rcp_rep = scr_pool.tile([128, QB], F32, tag="rcprep",
                                        name=f"rcprep{b}")
                nc.gpsimd.partition_broadcast(rcp_rep[:, :], rcprow[:, :])
                quarters[nb] = alloc_quarters(nb)
                s_pair(nb, 0)
                ot = ot_pool.tile([128, 2, QB], F8, tag="ot", name=f"ot{b}")
                ops0 = acc_ps.tile([128, QB], F32, tag="acc")
                pv_part(b, 0, 0, 8, ops0)
                s_pair(nb, 1)
                pv_part(b, 0, 8, 16, ops0)
                nc.vector.tensor_copy(ot[:, 0, :], ops0[:, :])
                s_pair(nb, 2)
                ops1 = acc_ps.tile([128, QB], F32, tag="acc")
                pv_part(b, 1, 0, 8, ops1)
                s_pair(nb, 3)
                pv_part(b, 1, 8, 16, ops1)
                nc.vector.tensor_copy(ot[:, 1, :], ops1[:, :])
                s_pair(nb, 4)
                y_b = outproj_y(b, ot, rcp_rep)
                s_pair(nb, 5)
                murow, mu_rep = stats_ln_a(b, y_b)
                s_pair(nb, 6)
                if b == 2:
                    # pre-run block 3's PV while its exps are landing
                    pv3["ops0"] = acc_ps.tile([128, QB], F32, tag="acc",
                                              name="pv3c0")
                    pv_part(3, 0, 0, 8, pv3["ops0"])
                rs_rep = stats_ln_b(b, y_b, murow)
                s_pair(nb, 7)
                if b == 2:
                    pv_part(3, 0, 8, 12, pv3["ops0"])
                    pv3["ops1"] = acc_ps.tile([128, QB], F32, tag="acc",
                                              name="pv3c1")
                    pv_part(3, 1, 0, 8, pv3["ops1"])
                dps[nb] = denom_part(nb, 0, 12)
                stats_ln_c(b, y_b, mu_rep, rs_rep)
                if b == 2:
                    pv_part(3, 1, 8, 12, pv3["ops1"])
                denom_part(nb, 12, 16, dps[nb])

            # ---------------- tail: block 3 ----------------
            b = 3
            # recip on the now-idle ACT; dps[3] is complete
            lnd = row_pool.tile([1, QB], F32, tag="lnd")
            nc.scalar.activation(out=lnd[:, :], in_=dps[3][:, :], func=AF.Ln)
            rcprow3 = row_pool.tile([1, QB], F32, tag="rcprow3")
            nc.scalar.activation(out=rcprow3[:, :], in_=lnd[:, :],
                                 func=AF.Exp, scale=-1.0)
            rcp_rep3 = scr_pool.tile([128, QB], F32, tag="rcprep",
                                     name="rcprep3")
            nc.gpsimd.partition_broadcast(rcp_rep3[:, :], rcprow3[:, :])
            ot = ot_pool.tile([128, 2, QB], F8, tag="ot", name="ot3")
            pv_part(3, 0, 12, 16, pv3["ops0"])
            nc.scalar.activation(out=ot[:, 0, :], in_=pv3["ops0"][:, :],
                                 func=AF.Copy)
            pv_part(3, 1, 12, 16, pv3["ops1"])
            nc.scalar.activation(out=ot[:, 1, :], in_=pv3["ops1"][:, :],
                                 func=AF.Copy)
            # tail halves: emit both outprojs first, then the LN stages
            # interleaved, so no half's matmuls park behind the other's chain
            ys = {}
            for half in range(2):
                ys[half] = outproj_y(b, ot, rcp_rep3, qo=half * 256, ql=256)
            stats_ln_last(b, ys, nh=2, ql=256)

    # Force Exp and Ln to resolve to the one table set containing both
    # (the default chooser alternates exp_and_others <-> natural_log_exp,
    # paying a ~1.3us table load per switch, ~17 loads per kernel).
    import bass_rust as _br
    from concourse.hw_specs import get_activation_tables as _gat

    def _patched_act_loads():
        has_act = any(
            isinstance(i, mybir.InstActivation)
            for blk in nc.main_func.blocks for i in blk.instructions
        )
        if not has_act:
            return
        tables = []
        for name, fns in _gat(nc.m.arch).items():
            if name != "natural_log_exp_and_others":
                fns = fns - {AF.Exp, AF.Ln}
            tables.append((name, fns))
        _br.insert_act_table_loads(nc, tables)

    nc.insert_act_table_loads = _patched_act_loads
    nc.compile()
    return nc


def get_nc():
    if "nc" not in _CACHE:
        _CACHE["nc"] = _build_nc()
    return _CACHE["nc"]


def make_in_maps(low, high, q_w, q_b, k_w, k_b, v_w, v_b, o_w, o_b, ln_g, ln_b):
    low_r = np.asarray(low, np.float32).reshape(B, C, N)
    high_r = np.asarray(high, np.float32).reshape(B, C, N)
    f32 = lambda x: np.ascontiguousarray(np.asarray(x, np.float32))
    f8 = lambda x: np.ascontiguousarray(
        np.asarray(x, np.float32).astype(ml_dtypes.float8_e4m3)
    )
    # v-bias is exactly equivalent to an out-proj bias shift because the
    # softmax rows sum to one: attn @ (V + 1 vb^T) @ o_w^T = attn @ V @ o_w^T
    # + (o_w @ v_b)^T, so fold it on the host.
    ob_eff = np.asarray(o_b, np.float32) + np.asarray(o_w, np.float32) @ np.asarray(v_b, np.float32)
    pv_cols = []
    for v in [np.asarray(q_b, np.float32) * QK_PRE,
              np.asarray(k_b, np.float32) * QK_PRE,
              ob_eff, ln_g, ln_b]:
        pv_cols.append(np.asarray(v, np.float32).reshape(2, 128).T)
    shared = {
        "wq": f8(np.asarray(q_w, np.float32).T * QK_PRE),
        "wk": f8(np.asarray(k_w, np.float32).T * QK_PRE),
        "wv": f8(np.asarray(v_w, np.float32).T),
        "wo": f8(np.asarray(o_w, np.float32).T),
        "pvec": f32(np.concatenate(pv_cols, axis=1)),
    }
    in_maps = []
    for i in range(8):
        bidx, h = i // 2, i % 2
        lo = low_r[bidx][:, h * NQ:(h + 1) * NQ]
        in_maps.append({
            "low": f32(lo),
            "lowf8": f8(lo),
            "high": f8(high_r[bidx]),
            **shared,
        })
    return in_maps


def assemble(results):
    out = np.empty((B, C, N), np.float32)
    for i in range(8):
        bidx, h = i // 2, i % 2
        out[bidx][:, h * NQ:(h + 1) * NQ] = results[i]["out"]
    return out.reshape(B, C, 64, 64)


def kernel(**inputs) -> np.ndarray:
    nc = get_nc()
    in_maps = make_in_maps(**inputs)
    res = run_bass_kernel_spmd(nc, in_maps, core_ids=list(range(8)))
    return assemble(res.results)


if __name__ == "__main__":
    pass


# revision 15
# speedup vs baseline: 1.1212x; 1.1212x over previous
"""ContentGuidedAttention Trainium2 kernel.

Full NxN single-head cross-attention + out-proj + residual + LayerNorm,
for B=4, C=256, H=W=64 (N=4096 tokens), distributed over 8 NeuronCores:
core i handles batch i//2, query-half i%2 (2048 queries, all 4096 keys).
No collectives: K/V are computed redundantly on the two cores sharing a
batch (~5% extra FLOPs).

Layout strategy (channel-major, zero transposes, fp8 DoubleRow on every
matmul the PE streams):
  - Q^T/K^T as [C, n] fp8e4 (q/k weights prescaled by 16, compensated in
    the softmax exp scale); V token-major [n, C] fp8e4 via DR matmuls,
    evacuated on GpSimd (DVE is the preamble bottleneck)
  - S^T = K Q^T fp8 DR; exp on ACT -> P^T fp8e4
  - softmax denominator: DR ones-vector matmuls accumulate into a [1, q]
    psum row; 1/denom via a single DVE reciprocal_approx_fast (keeps the
    ACT queue exp-only), then gpsimd partition_broadcast
  - PV: O^T[c, q] = sum_k V[k,c] P^T[k,q], fp8 DR; O^T evacuated to fp8
    so the out-proj is DR too; residual uses a separate f32 copy of low
  - scheduling: the steady-state loop WEAVES 2-si S-matmul pairs between
    ~1.5us chunks of PV/outproj/LN/denominator matmuls so the in-order
    PE queue never parks behind an S matmul whose psum buffer is still
    being drained by ACT (st_ps has only 2 bufs), and ACT always has exp
    work queued.  Same fine-grained interleave in the projection
    preamble.  ~35 dummy 128-col matmuls at t~3.5us warm the PE HAM
    clock-gate before real work; inputs ride 4 DMA queues in parallel.
  - tail: last block's out-proj + LN run as two 256-query halves so the
    serial ACT/DVE chain of one half overlaps the other's matmuls.
"""

import ml_dtypes
import numpy as np

import concourse.bass as bass
import concourse.mybir as mybir
import concourse.tile as tile
from concourse import bacc
from concourse.bass import ds, ts
from concourse.bass_utils import run_bass_kernel_spmd

F32 = mybir.dt.float32
F32R = mybir.dt.float32r
BF16 = mybir.dt.bfloat16
F8 = mybir.dt.float8e4
AF = mybir.ActivationFunctionType
OP = mybir.AluOpType
DR = mybir.MatmulPerfMode.DoubleRow

B = 4
C = 256
N = 4096          # tokens per batch
NQ = 2048         # queries per core
QB = 512          # query block
NQB = NQ // QB    # 4
NKC = N // 128    # 32 key chunks
NKR = 4           # key ranges (1024 keys each) for K^T / V tiles
QK_PRE = 16.0     # host-side prescale on q/k weights (fp8 range centering)
SCALE = (C // 8) ** -0.5
EXP_SCALE = SCALE / (QK_PRE * QK_PRE)
LN_EPS = 1e-5

_CACHE = {}


def _build_nc():
    nc = bacc.Bacc("TRN2", target_bir_lowering=False, debug=False)

    low_d = nc.declare_dram_parameter("low", [C, NQ], F32R, isOutput=False)
    lowf8_d = nc.declare_dram_parameter("lowf8", [C, NQ], F8, isOutput=False)
    high_d = nc.declare_dram_parameter("high", [C, N], F8, isOutput=False)
    # weights are passed pre-transposed: [c_in, c_out], fp8
    wq_d = nc.declare_dram_parameter("wq", [C, C], F8, isOutput=False)
    wk_d = nc.declare_dram_parameter("wk", [C, C], F8, isOutput=False)
    wv_d = nc.declare_dram_parameter("wv", [C, C], F8, isOutput=False)
    wo_d = nc.declare_dram_parameter("wo", [C, C], F8, isOutput=False)
    # qb, kb, ob, lng, lnb prepacked host-side as [128, 10]
    pvec_d = nc.declare_dram_parameter("pvec", [128, 10], F32, isOutput=False)
    out_d = nc.declare_dram_parameter("out", [C, NQ], F32, isOutput=True)

    with tile.TileContext(nc) as tc:
        with (
            tc.tile_pool(name="persist", bufs=1) as pp,
            tc.tile_pool(name="high", bufs=4) as high_pool,
            tc.tile_pool(name="pt", bufs=8) as pt_pool,
            tc.tile_pool(name="ot", bufs=3) as ot_pool,
            tc.tile_pool(name="scratch", bufs=3) as scr_pool,
            tc.tile_pool(name="rowscr", bufs=1) as row_pool,
            tc.tile_pool(name="outsb", bufs=4) as out_pool,
            tc.tile_pool(name="st_ps", bufs=2, space="PSUM") as st_ps,
            tc.tile_pool(name="acc_ps", bufs=3, space="PSUM") as acc_ps,
            tc.tile_pool(name="row_ps", bufs=1, space="PSUM") as row_ps,
        ):
            # ---------------- constants + PE warm-up ----------------
            stage = pp.tile([128, 128], F32)
            nc.vector.memset(stage[:, :], 1.0)
            ones128 = pp.tile([128, 1], F32R)    # partition-reduce lhsT
            nc.vector.tensor_copy(ones128[:, :], stage[:, 0:1])
            # tiny exp: pulls the ACT table load to ~3.6us (ACT idle)
            tinyrow = pp.tile([1, 1], F32)
            nc.scalar.activation(out=tinyrow[:, :], in_=stage[0:1, 0:1],
                                 func=AF.Exp)
            # ~35 dummy matmuls release the PE HAM clock-gate (~3.4us of
            # sustained activity) before the first real matmul at ~8us
            warm_ps = row_ps.tile([1, 128], F32, tag="row")
            for w in range(35):
                nc.tensor.matmul(
                    out=warm_ps[:, :], lhsT=ones128[:, :],
                    rhs=stage[:, :].bitcast(F32R),
                    start=True, stop=True, skip_group_check=True,
                )
            ones2f8 = pp.tile([128, 2, 16], F8)  # DoubleRow denom lhsT
            nc.vector.tensor_copy(ones2f8[:, :, 0], stage[:, 0:2])
            ones_col = pp.tile([1, 128], F32R)   # K=1 row-broadcast lhsT
            nc.vector.tensor_copy(ones_col[:, :], stage[0:1, :])
            epsb = pp.tile([1, 1], F32)          # LN epsilon bias
            nc.vector.memset(epsb[:, :], LN_EPS)

            # ---------------- input DMAs on 4 parallel queues --------
            wk_sb = pp.tile([128, 2, C], F8)
            wv_sb = pp.tile([128, 2, C], F8)
            wq_sb = pp.tile([128, 2, C], F8)
            wo_sb = pp.tile([128, 2, C], F8)
            pvec = pp.tile([128, 10], F32)
            lowf8_sb = pp.tile([128, 2, NQ], F8)
            low_sb = pp.tile([128, 2, NQ], F32R)
            for j in range(2):
                nc.scalar.dma_start(out=wk_sb[:, j, :], in_=wk_d[ds(j * 128, 128), :])
            nc.gpsimd.dma_start(out=pvec[:, :], in_=pvec_d[:, :])
            for j in range(2):
                nc.gpsimd.dma_start(out=wq_sb[:, j, :], in_=wq_d[ds(j * 128, 128), :])
            for j in range(2):
                nc.gpsimd.dma_start(out=wv_sb[:, j, :], in_=wv_d[ds(j * 128, 128), :])
            for j in range(2):
                nc.gpsimd.dma_start(out=wo_sb[:, j, :], in_=wo_d[ds(j * 128, 128), :])
            hi_tiles = [
                high_pool.tile([128, 2, 1024], F8, name=f"hi{r}")
                for r in range(NKR)
            ]
            # range 0 rides first, split in h-halves so the very first
            # K-projection matmul waits on a [128,512] transfer only
            for h in range(2):
                for j in range(2):
                    nc.sync.dma_start(
                        out=hi_tiles[0][:, j, ds(h * 512, 512)],
                        in_=high_d[ds(j * 128, 128), ds(h * 512, 512)],
                    )
            for j in range(2):
                nc.sync.dma_start(out=lowf8_sb[:, j, :], in_=lowf8_d[ds(j * 128, 128), :])
            for r in range(1, NKR):
                for j in range(2):
                    nc.sync.dma_start(
                        out=hi_tiles[r][:, j, :],
                        in_=high_d[ds(j * 128, 128), ds(r * 1024, 1024)],
                    )
            for j in range(2):
                nc.sync.dma_start(out=low_sb[:, j, :], in_=low_d[ds(j * 128, 128), :])

            QBIAS, KBIAS, OBIAS, LNG, LNB = 0, 2, 4, 6, 8

            kt_sb = [
                pp.tile([128, 2, 1024], F8, name=f"kt{r}", tag=f"kt{r}")
                for r in range(NKR)
            ]
            v_sb = [
                pp.tile([128, 8, C], F8, name=f"v{r}", tag=f"v{r}")
                for r in range(NKR)
            ]
            qt_all = pp.tile([128, 2, NQ], F8)

            # ---------------- work units ----------------
            def k_unit(r, h, split_j=False):
                # K^T: out [cout, k] = sum_cin wk[cin, cout] high[cin, k]
                for c in range(2):
                    kps = acc_ps.tile([128, 512], F32, tag="acc")
                    if split_j:
                        # first matmuls only need the first DMA chunks
                        for j in range(2):
                            nc.tensor.matmul(
                                out=kps[:, :],
                                lhsT=wk_sb[:, j, ds(c * 128, 128)],
                                rhs=hi_tiles[r][:, j, ds(h * 512, 512)],
                                start=(j == 0), stop=(j == 1),
                            )
                    else:
                        nc.tensor.matmul(
                            out=kps[:, :],
                            lhsT=wk_sb[:, :, ds(c * 128, 128)],
                            rhs=hi_tiles[r][:, :, ds(h * 512, 512)],
                            start=True, stop=True,
                            perf_mode=DR,
                        )
                    # K bias dropped: a k-independent logit shift per query,
                    # exactly cancelled by softmax
                    nc.vector.tensor_copy(
                        kt_sb[r][:, c, ds(h * 512, 512)], kps[:, :]
                    )

            def v_unit(r, up):
                # V: out [k, cout] = sum_cin high[cin, k] wv[cin, cout]
                # DR over the cin halves; last range evacuates on ACT to
                # balance the preamble DVE load
                vps = acc_ps.tile([128, 2, C], F32, tag="acc")
                for i in range(2):
                    u = up * 2 + i
                    nc.tensor.matmul(
                        out=vps[:, i, :],
                        lhsT=hi_tiles[r][:, :, ds(u * 128, 128)],
                        rhs=wv_sb[:, :, :],
                        start=True, stop=True,
                        perf_mode=DR,
                    )
                if r >= 2:
                    nc.scalar.activation(
                        out=v_sb[r][:, ds(up * 2, 2), :], in_=vps[:, :, :],
                        func=AF.Copy,
                    )
                else:
                    nc.vector.tensor_copy(
                        v_sb[r][:, ds(up * 2, 2), :], vps[:, :, :]
                    )

            def q_proj(qb4):
                for c in range(2):
                    qps = acc_ps.tile([128, QB], F32, tag="acc")
                    nc.tensor.matmul(
                        out=qps[:, :],
                        lhsT=wq_sb[:, :, ds(c * 128, 128)],
                        rhs=lowf8_sb[:, :, ds(qb4 * QB, QB)],
                        start=True, stop=True,
                        perf_mode=DR,
                    )
                    nc.vector.tensor_scalar_add(
                        out=qt_all[:, c, ds(qb4 * QB, QB)], in0=qps[:, :],
                        scalar1=pvec[:, ds(QBIAS + c, 1)],
                    )

            def alloc_quarters(b):
                return [
                    pt_pool.tile([128, 8, QB], F8, tag="ptq", name=f"ptq{g}")
                    for g in range(4)
                ]

            quarters = {}

            def s_single(b, si):
                # 1 si = 2 key chunks: 2 S matmuls + 1 exp
                qsl = ds(b * QB, QB)
                sps = st_ps.tile([128, 2, QB], F32, tag="st")
                for u in range(2):
                    kc = si * 2 + u
                    nc.tensor.matmul(
                        out=sps[:, u, :],
                        lhsT=kt_sb[kc // 8][:, :, ds((kc % 8) * 128, 128)],
                        rhs=qt_all[:, :, qsl],
                        start=True, stop=True,
                        perf_mode=DR,
                    )
                nc.scalar.activation(
                    out=quarters[b][si // 4][:, ds((si % 4) * 2, 2), :],
                    in_=sps[:, :, :],
                    func=AF.Exp,
                    scale=EXP_SCALE,
                )

            def s_pair(b, p):
                s_single(b, 2 * p)
                s_single(b, 2 * p + 1)

            def denom_part(b, t0, t1, dps=None):
                # split accumulation: t12-15 can be emitted after other PE
                # work so the last exps of block b have time to land
                if dps is None:
                    dps = row_ps.tile([1, QB], F32, tag="row")
                for t in range(t0, t1):
                    nc.tensor.matmul(
                        out=dps[:, :],
                        lhsT=ones2f8[:, :, 0:1],
                        rhs=quarters[b][t // 4][:, ds((t % 4) * 2, 2), :],
                        start=(t == 0), stop=(t == t1 - 1),
                        perf_mode=DR,
                        skip_group_check=True,
                    )
                return dps

            def pv_part(b, c, t0, t1, ops):
                for t in range(t0, t1):
                    nc.tensor.matmul(
                        out=ops[:, :],
                        lhsT=v_sb[t // 4][:, ds((t % 4) * 2, 2),
                                         ds(c * 128, 128)],
                        rhs=quarters[b][t // 4][:, ds((t % 4) * 2, 2), :],
                        start=(t == 0), stop=(t == t1 - 1),
                        perf_mode=DR,
                        skip_group_check=True,
                    )

            def outproj_y(b, ot, rcp_rep, qo=0, ql=QB):
                qsl = ds(b * QB + qo, ql)
                y_sb = ot_pool.tile([128, 2, ql], F32R, tag="y",
                                    name=f"y{b}_{qo}")
                for c in range(2):
                    pps = acc_ps.tile([128, ql], F32, tag="acc")
                    nc.tensor.matmul(
                        out=pps[:, :],
                        lhsT=wo_sb[:, :, ds(c * 128, 128)],
                        rhs=ot[:, :, ds(qo, ql)],
                        start=True, stop=True,
                        perf_mode=DR,
                    )
                    ysc = scr_pool.tile([128, ql], F32, tag="scr")
                    nc.vector.tensor_mul(
                        out=ysc[:, :], in0=pps[:, :], in1=rcp_rep[:, ds(qo, ql)]
                    )
                    nc.vector.scalar_tensor_tensor(
                        out=y_sb[:, c, :],
                        in0=ysc[:, :],
                        scalar=pvec[:, ds(OBIAS + c, 1)],
                        in1=low_sb[:, c, qsl].bitcast(F32),
                        op0=OP.add, op1=OP.add,
                    )
                return y_sb

            def stats_ln_a(b, y_sb):
                sy_ps = row_ps.tile([1, QB], F32, tag="row")
                for c in range(2):
                    nc.tensor.matmul(
                        out=sy_ps[:, :], lhsT=ones128[:, :],
                        rhs=y_sb[:, c, :], start=(c == 0), stop=(c == 1),
                    )
                murow = row_pool.tile([1, QB], F32, tag="murow")
                nc.vector.tensor_scalar_mul(
                    out=murow[:, :], in0=sy_ps[:, :], scalar1=1.0 / C
                )
                mu_rep = scr_pool.tile([128, QB], F32, tag="murep")
                nc.gpsimd.partition_broadcast(mu_rep[:, :], murow[:, :])
                return murow, mu_rep

            def stats_ln_b(b, y_sb, murow):
                sy2_ps = row_ps.tile([1, QB], F32, tag="row")
                for c in range(2):
                    ysq = scr_pool.tile([128, QB], F32R, tag="ysq")
                    nc.vector.tensor_mul(
                        out=ysq[:, :],
                        in0=y_sb[:, c, :].bitcast(F32),
                        in1=y_sb[:, c, :].bitcast(F32),
                    )
                    nc.tensor.matmul(
                        out=sy2_ps[:, :], lhsT=ones128[:, :],
                        rhs=ysq[:, :], start=(c == 0), stop=(c == 1),
                    )
                # C*var = sy2 - C*mu^2 ; rstd = exp(-0.5 ln((C var)/C + eps))
                mu2row = row_pool.tile([1, QB], F32, tag="mu2row")
                nc.vector.tensor_mul(
                    out=mu2row[:, :], in0=murow[:, :], in1=murow[:, :],
                )
                varrow = row_pool.tile([1, QB], F32, tag="varrow")
                nc.vector.scalar_tensor_tensor(
                    out=varrow[:, :], in0=mu2row[:, :], scalar=-float(C),
                    in1=sy2_ps[:, :], op0=OP.mult, op1=OP.add,
                )
                lnv = row_pool.tile([1, QB], F32, tag="lnv")
                nc.scalar.activation(
                    out=lnv[:, :], in_=varrow[:, :], func=AF.Ln,
                    scale=1.0 / C, bias=epsb[:, :],
                )
                rstdrow = row_pool.tile([1, QB], F32, tag="rstdrow")
                nc.scalar.activation(
                    out=rstdrow[:, :], in_=lnv[:, :], func=AF.Exp, scale=-0.5
                )
                rs_rep = scr_pool.tile([128, QB], F32, tag="rsrep")
                nc.gpsimd.partition_broadcast(rs_rep[:, :], rstdrow[:, :])
                return rs_rep

            def stats_ln_c(b, y_sb, mu_rep, rs_rep):
                qsl = ds(b * QB, QB)
                for c in range(2):
                    yn = scr_pool.tile([128, QB], F32, tag="scr")
                    nc.vector.tensor_sub(
                        out=yn[:, :],
                        in0=y_sb[:, c, :].bitcast(F32),
                        in1=mu_rep[:, :],
                    )
                    nc.vector.tensor_mul(
                        out=yn[:, :], in0=yn[:, :], in1=rs_rep[:, :]
                    )
                    osb = out_pool.tile([128, QB], F32)
                    nc.vector.tensor_scalar(
                        out=osb[:, :], in0=yn[:, :],
                        scalar1=pvec[:, ds(LNG + c, 1)],
                        scalar2=pvec[:, ds(LNB + c, 1)],
                        op0=OP.mult, op1=OP.add,
                    )
                    nc.sync.dma_start(
                        out=out_d[ds(c * 128, 128), qsl], in_=osb[:, :]
                    )

            def stats_ln_last(b, ys, nh, ql):
                # span-critical tail, nh halves stage-interleaved so each
                # engine queue alternates halves and the serial chain of
                # one half hides behind the matmuls of the next.  murow on
                # ACT; rstd broadcast via a K=1 PE matmul into psum.
                sy_ps, murow, sy2_ps = {}, {}, {}
                mu2row, varrow, lnv, rstdrow = {}, {}, {}, {}
                mu_rep, rs_ps = {}, {}
                for h in range(nh):
                    sy_ps[h] = st_ps.tile([1, ql], F32, tag="st",
                                          name=f"syp{h}")
                    for c in range(2):
                        nc.tensor.matmul(
                            out=sy_ps[h][:, :], lhsT=ones128[:, :],
                            rhs=ys[h][:, c, :], start=(c == 0), stop=(c == 1),
                        )
                    murow[h] = row_pool.tile([1, ql], F32, tag="murow",
                                             name=f"mur{h}")
                    nc.scalar.activation(
                        out=murow[h][:, :], in_=sy_ps[h][:, :], func=AF.Copy,
                        scale=1.0 / C,
                    )
                    sy2_ps[h] = st_ps.tile([1, ql], F32, tag="st",
                                           name=f"sy2p{h}")
                    for c in range(2):
                        ysq = scr_pool.tile([128, ql], F32R, tag="ysq")
                        nc.vector.tensor_mul(
                            out=ysq[:, :],
                            in0=ys[h][:, c, :].bitcast(F32),
                            in1=ys[h][:, c, :].bitcast(F32),
                        )
                        nc.tensor.matmul(
                            out=sy2_ps[h][:, :], lhsT=ones128[:, :],
                            rhs=ysq[:, :], start=(c == 0), stop=(c == 1),
                        )
                for h in range(nh):
                    mu2row[h] = row_pool.tile([1, ql], F32, tag="mu2row",
                                              name=f"mu2r{h}")
                    nc.vector.tensor_mul(
                        out=mu2row[h][:, :], in0=murow[h][:, :],
                        in1=murow[h][:, :],
                    )
                    varrow[h] = row_pool.tile([1, ql], F32, tag="varrow",
                                              name=f"varr{h}")
                    nc.vector.scalar_tensor_tensor(
                        out=varrow[h][:, :], in0=mu2row[h][:, :],
                        scalar=-float(C),
                        in1=sy2_ps[h][:, :], op0=OP.mult, op1=OP.add,
                    )
                    mu_rep[h] = scr_pool.tile([128, ql], F32, tag="murep",
                                              name=f"murep{h}")
                    nc.gpsimd.partition_broadcast(mu_rep[h][:, :],
                                                  murow[h][:, :])
                for h in range(nh):
                    lnv[h] = row_pool.tile([1, ql], F32, tag="lnv",
                                           name=f"lnv{h}")
                    nc.scalar.activation(
                        out=lnv[h][:, :], in_=varrow[h][:, :], func=AF.Ln,
                        scale=1.0 / C, bias=epsb[:, :],
                    )
                    rstdrow[h] = row_pool.tile([1, ql], F32R, tag="rstdrow",
                                               name=f"rstdr{h}")
                    nc.scalar.activation(
                        out=rstdrow[h][:, :], in_=lnv[h][:, :], func=AF.Exp,
                        scale=-0.5,
                    )
                    rs_ps[h] = acc_ps.tile([128, ql], F32, tag="acc",
                                           name=f"rsps{h}")
                    nc.tensor.matmul(
                        out=rs_ps[h][:, :], lhsT=ones_col[:, :],
                        rhs=rstdrow[h][:, :], start=True, stop=True,
                    )
                for h in range(nh):
                    qsl = ds(b * QB + h * ql, ql)
                    for c in range(2):
                        yn = scr_pool.tile([128, ql], F32, tag="scr")
                        nc.vector.tensor_sub(
                            out=yn[:, :],
                            in0=ys[h][:, c, :].bitcast(F32),
                            in1=mu_rep[h][:, :],
                        )
                        nc.vector.tensor_mul(
                            out=yn[:, :], in0=yn[:, :], in1=rs_ps[h][:, :]
                        )
                        osb = out_pool.tile([128, ql], F32)
                        nc.vector.tensor_scalar(
                            out=osb[:, :], in0=yn[:, :],
                            scalar1=pvec[:, ds(LNG + c, 1)],
                            scalar2=pvec[:, ds(LNB + c, 1)],
                            op0=OP.mult, op1=OP.add,
                        )
                        nc.sync.dma_start(
                            out=out_d[ds(c * 128, 128), qsl], in_=osb[:, :]
                        )

            # ---------------- preamble: projections woven 1:1 with
            # block-0 S singles so the ACT exp stream starts ~10us in and
            # never starves, and no S matmul parks the in-order PE queue
            # (its psum buffer is 2 exps back, covered by a heavy unit)
            quarters[0] = alloc_quarters(0)
            k_unit(0, 0, split_j=True)
            q_proj(0)
            k_unit(0, 1)
            heavies = (
                [(k_unit, 1, 0), (k_unit, 1, 1), (k_unit, 2, 0),
                 (k_unit, 2, 1), (k_unit, 3, 0), (k_unit, 3, 1)]
                + [(v_unit, r, up) for r in range(3) for up in range(4)
                   ][:10]
            )
            for i in range(16):
                fn, a0, a1 = heavies[i]
                fn(a0, a1)
                s_single(0, i)
            v_unit(2, 2)
            q_proj(1)
            quarters[1] = alloc_quarters(1)
            s_single(1, 0)
            v_unit(2, 3)
            s_single(1, 1)
            v_unit(3, 0)
            s_single(1, 2)
            v_unit(3, 1)
            s_single(1, 3)
            v_unit(3, 2)
            s_single(1, 4)
            v_unit(3, 3)
            s_single(1, 5)
            q_proj(2)
            q_proj(3)
            dps = {0: denom_part(0, 0, 16)}

            # ---------------- steady state (b = 0..2) ----------------
            # block nb's S singles woven between ~1-3us chunks of block
            # b's PV/outproj/LN/denominator matmuls; block 3's PV t0-11
            # pre-woven into b=2 so the final iteration is tiny
            pv3 = {}
            sched = {
                0: [[6], [7], [8], [9], [10], [11], [12], [13]],
                1: [[0, 1], [2, 3], [4, 5], [6, 7], [8, 9], [10, 11],
                    [12], [13]],
                2: [[0, 1], [2, 3], [4, 5], [6, 7], [8, 9], [10, 11],
                    [12], [13]],
            }
            for b in range(NQB - 1):
                nb = b + 1
                # 1/denom: single custom-DVE op (ACT stays exp-only)
                rcprow = row_pool.tile([1, QB], F32, tag="rcprow",
                                       name=f"rcprow{b}")
                nc.vector.reciprocal_approx_fast(
                    out=rcprow[:, :], in_=dps[b][:, :]
                )
                rcp_rep = scr_pool.tile([128, QB], F32, tag="rcprep",
                                        name=f"rcprep{b}")
                nc.gpsimd.partition_broadcast(rcp_rep[:, :], rcprow[:, :])
                if b >= 1:
                    quarters[nb] = alloc_quarters(nb)
                Wl = sched[b]

                def weave(units, nb=nb):
                    for si in units:
                        s_single(nb, si)

                weave(Wl[0])
                ot = ot_pool.tile([128, 2, QB], F8, tag="ot", name=f"ot{b}")
                ops0 = acc_ps.tile([128, QB], F32, tag="acc")
                pv_part(b, 0, 0, 8, ops0)
                weave(Wl[1])
                pv_part(b, 0, 8, 16, ops0)
                nc.vector.tensor_copy(ot[:, 0, :], ops0[:, :])
                weave(Wl[2])
                ops1 = acc_ps.tile([128, QB], F32, tag="acc")
                pv_part(b, 1, 0, 8, ops1)
                weave(Wl[3])
                pv_part(b, 1, 8, 16, ops1)
                nc.vector.tensor_copy(ot[:, 1, :], ops1[:, :])
                weave(Wl[4])
                y_b = outproj_y(b, ot, rcp_rep)
                weave(Wl[5])
                murow, mu_rep = stats_ln_a(b, y_b)
                weave(Wl[6])
                if b == 2:
                    # pre-run block 3's PV while its exps are landing
                    pv3["ops0"] = acc_ps.tile([128, QB], F32, tag="acc",
                                              name="pv3c0")
                    pv_part(3, 0, 0, 8, pv3["ops0"])
                rs_rep = stats_ln_b(b, y_b, murow)
                weave(Wl[7])
                if b == 2:
                    pv_part(3, 0, 8, 12, pv3["ops0"])
                    pv3["ops1"] = acc_ps.tile([128, QB], F32, tag="acc",
                                              name="pv3c1")
                    pv_part(3, 1, 0, 8, pv3["ops1"])
                s_single(nb, 14)
                dps[nb] = denom_part(nb, 0, 15)
                stats_ln_c(b, y_b, mu_rep, rs_rep)
                s_single(nb, 15)
                if b == 2:
                    pv_part(3, 1, 8, 12, pv3["ops1"])
                denom_part(nb, 15, 16, dps[nb])

            # ---------------- tail: block 3 ----------------
            b = 3
            # recip on the now-idle ACT; dps[3] is complete
            lnd = row_pool.tile([1, QB], F32, tag="lnd")
            nc.scalar.activation(out=lnd[:, :], in_=dps[3][:, :], func=AF.Ln)
            rcprow3 = row_pool.tile([1, QB], F32, tag="rcprow3")
            nc.scalar.activation(out=rcprow3[:, :], in_=lnd[:, :],
                                 func=AF.Exp, scale=-1.0)
            rcp_rep3 = scr_pool.tile([128, QB], F32, tag="rcprep",
                                     name="rcprep3")
            nc.gpsimd.partition_broadcast(rcp_rep3[:, :], rcprow3[:, :])
            ot = ot_pool.tile([128, 2, QB], F8, tag="ot", name="ot3")
            pv_part(3, 0, 12, 16, pv3["ops0"])
            nc.scalar.activation(out=ot[:, 0, :], in_=pv3["ops0"][:, :],
                                 func=AF.Copy)
            pv_part(3, 1, 12, 16, pv3["ops1"])
            nc.scalar.activation(out=ot[:, 1, :], in_=pv3["ops1"][:, :],
                                 func=AF.Copy)
            # tail halves: emit both outprojs first, then the LN stages
            # interleaved, so no half's matmuls park behind the other's chain
            ys = {}
            for half in range(2):
                ys[half] = outproj_y(b, ot, rcp_rep3, qo=half * 256, ql=256)
            stats_ln_last(b, ys, nh=2, ql=256)

    # Force Exp and Ln to resolve to the one table set containing both
    # (the default chooser alternates exp_and_others <-> natural_log_exp,
    # paying a ~1.3us table load per switch, ~17 loads per kernel).
    import bass_rust as _br
    from concourse.hw_specs import get_activation_tables as _gat

    def _patched_act_loads():
        has_act = any(
            isinstance(i, mybir.InstActivation)
            for blk in nc.main_func.blocks for i in blk.instructions
        )
        if not has_act:
            return
        tables = []
        for name, fns in _gat(nc.m.arch).items():
            if name != "natural_log_exp_and_others":
                fns = fns - {AF.Exp, AF.Ln}
            tables.append((name, fns))
        _br.insert_act_table_loads(nc, tables)

    nc.insert_act_table_loads = _patched_act_loads
    nc.compile()
    return nc


def get_nc():
    if "nc" not in _CACHE:
        _CACHE["nc"] = _build_nc()
    return _CACHE["nc"]


def make_in_maps(low, high, q_w, q_b, k_w, k_b, v_w, v_b, o_w, o_b, ln_g, ln_b):
    low_r = np.asarray(low, np.float32).reshape(B, C, N)
    high_r = np.asarray(high, np.float32).reshape(B, C, N)
    f32 = lambda x: np.ascontiguousarray(np.asarray(x, np.float32))
    f8 = lambda x: np.ascontiguousarray(
        np.asarray(x, np.float32).astype(ml_dtypes.float8_e4m3)
    )
    # v-bias is exactly equivalent to an out-proj bias shift because the
    # softmax rows sum to one: attn @ (V + 1 vb^T) @ o_w^T = attn @ V @ o_w^T
    # + (o_w @ v_b)^T, so fold it on the host.
    ob_eff = np.asarray(o_b, np.float32) + np.asarray(o_w, np.float32) @ np.asarray(v_b, np.float32)
    pv_cols = []
    for v in [np.asarray(q_b, np.float32) * QK_PRE,
              np.asarray(k_b, np.float32) * QK_PRE,
              ob_eff, ln_g, ln_b]:
        pv_cols.append(np.asarray(v, np.float32).reshape(2, 128).T)
    shared = {
        "wq": f8(np.asarray(q_w, np.float32).T * QK_PRE),
        "wk": f8(np.asarray(k_w, np.float32).T * QK_PRE),
        "wv": f8(np.asarray(v_w, np.float32).T),
        "wo": f8(np.asarray(o_w, np.float32).T),
        "pvec": f32(np.concatenate(pv_cols, axis=1)),
    }
    in_maps = []
    for i in range(8):
        bidx, h = i // 2, i % 2
        lo = low_r[bidx][:, h * NQ:(h + 1) * NQ]
        in_maps.append({
            "low": f32(lo),
            "lowf8": f8(lo),
            "high": f8(high_r[bidx]),
            **shared,
        })
    return in_maps


def assemble(results):
    out = np.empty((B, C, N), np.float32)
    for i in range(8):
        bidx, h = i // 2, i % 2
        out[bidx][:, h * NQ:(h + 1) * NQ] = results[i]["out"]
    return out.reshape(B, C, 64, 64)


def kernel(**inputs) -> np.ndarray:
    nc = get_nc()
    in_maps = make_in_maps(**inputs)
    res = run_bass_kernel_spmd(nc, in_maps, core_ids=list(range(8)))
    return assemble(res.results)


if __name__ == "__main__":
    pass


# revision 16
# speedup vs baseline: 1.1278x; 1.0059x over previous
"""ContentGuidedAttention Trainium2 kernel.

Full NxN single-head cross-attention + out-proj + residual + LayerNorm,
for B=4, C=256, H=W=64 (N=4096 tokens), distributed over 8 NeuronCores:
core i handles batch i//2, query-half i%2 (2048 queries, all 4096 keys).
No collectives: K/V are computed redundantly on the two cores sharing a
batch (~5% extra FLOPs).

Layout strategy (channel-major, zero transposes, fp8 DoubleRow on every
matmul the PE streams):
  - Q^T/K^T as [C, n] fp8e4 (q/k weights prescaled by 16, compensated in
    the softmax exp scale); V token-major [n, C] fp8e4 via DR matmuls,
    evacuated on GpSimd (DVE is the preamble bottleneck)
  - S^T = K Q^T fp8 DR; exp on ACT -> P^T fp8e4
  - softmax denominator: DR ones-vector matmuls accumulate into a [1, q]
    psum row; 1/denom via a single DVE reciprocal_approx_fast (keeps the
    ACT queue exp-only), then gpsimd partition_broadcast
  - PV: O^T[c, q] = sum_k V[k,c] P^T[k,q], fp8 DR; O^T evacuated to fp8
    so the out-proj is DR too; residual uses a separate f32 copy of low
  - scheduling: the steady-state loop WEAVES 2-si S-matmul pairs between
    ~1.5us chunks of PV/outproj/LN/denominator matmuls so the in-order
    PE queue never parks behind an S matmul whose psum buffer is still
    being drained by ACT (st_ps has only 2 bufs), and ACT always has exp
    work queued.  Same fine-grained interleave in the projection
    preamble.  ~35 dummy 128-col matmuls at t~3.5us warm the PE HAM
    clock-gate before real work; inputs ride 4 DMA queues in parallel.
  - tail: last block's out-proj + LN run as two 256-query halves so the
    serial ACT/DVE chain of one half overlaps the other's matmuls.
"""

import ml_dtypes
import numpy as np

import concourse.bass as bass
import concourse.mybir as mybir
import concourse.tile as tile
from concourse import bacc
from concourse.bass import ds, ts
from concourse.bass_utils import run_bass_kernel_spmd

F32 = mybir.dt.float32
F32R = mybir.dt.float32r
BF16 = mybir.dt.bfloat16
F8 = mybir.dt.float8e4
AF = mybir.ActivationFunctionType
OP = mybir.AluOpType
DR = mybir.MatmulPerfMode.DoubleRow

B = 4
C = 256
N = 4096          # tokens per batch
NQ = 2048         # queries per core
QB = 512          # query block
NQB = NQ // QB    # 4
NKC = N // 128    # 32 key chunks
NKR = 4           # key ranges (1024 keys each) for K^T / V tiles
QK_PRE = 16.0     # host-side prescale on q/k weights (fp8 range centering)
SCALE = (C // 8) ** -0.5
EXP_SCALE = SCALE / (QK_PRE * QK_PRE)
LN_EPS = 1e-5

_CACHE = {}


def _build_nc():
    nc = bacc.Bacc("TRN2", target_bir_lowering=False, debug=False)

    low_d = nc.declare_dram_parameter("low", [C, NQ], F32R, isOutput=False)
    lowf8_d = nc.declare_dram_parameter("lowf8", [C, NQ], F8, isOutput=False)
    high_d = nc.declare_dram_parameter("high", [C, N], F8, isOutput=False)
    # weights are passed pre-transposed: [c_in, c_out], fp8
    wq_d = nc.declare_dram_parameter("wq", [C, C], F8, isOutput=False)
    wk_d = nc.declare_dram_parameter("wk", [C, C], F8, isOutput=False)
    wv_d = nc.declare_dram_parameter("wv", [C, C], F8, isOutput=False)
    wo_d = nc.declare_dram_parameter("wo", [C, C], F8, isOutput=False)
    # qb, kb, ob, lng, lnb prepacked host-side as [128, 10]
    pvec_d = nc.declare_dram_parameter("pvec", [128, 10], F32, isOutput=False)
    out_d = nc.declare_dram_parameter("out", [C, NQ], F32, isOutput=True)

    with tile.TileContext(nc) as tc:
        with (
            tc.tile_pool(name="persist", bufs=1) as pp,
            tc.tile_pool(name="high", bufs=4) as high_pool,
            tc.tile_pool(name="pt", bufs=8) as pt_pool,
            tc.tile_pool(name="ot", bufs=3) as ot_pool,
            tc.tile_pool(name="scratch", bufs=3) as scr_pool,
            tc.tile_pool(name="rowscr", bufs=1) as row_pool,
            tc.tile_pool(name="outsb", bufs=4) as out_pool,
            tc.tile_pool(name="st_ps", bufs=2, space="PSUM") as st_ps,
            tc.tile_pool(name="acc_ps", bufs=3, space="PSUM") as acc_ps,
            tc.tile_pool(name="row_ps", bufs=1, space="PSUM") as row_ps,
        ):
            # ---------------- constants + PE warm-up ----------------
            stage = pp.tile([128, 128], F32)
            nc.vector.memset(stage[:, :], 1.0)
            ones128 = pp.tile([128, 1], F32R)    # partition-reduce lhsT
            nc.vector.tensor_copy(ones128[:, :], stage[:, 0:1])
            # tiny exp: pulls the ACT table load to ~3.6us (ACT idle)
            tinyrow = pp.tile([1, 1], F32)
            nc.scalar.activation(out=tinyrow[:, :], in_=stage[0:1, 0:1],
                                 func=AF.Exp)
            # ~10 dummy matmuls bridge the PE from the start barrier to
            # the first DMA landing; the real matmul stream then keeps the
            # HAM clock-gate window busy until it unthrottles (~11us)
            warm_ps = row_ps.tile([1, 128], F32, tag="row")
            for w in range(10):
                nc.tensor.matmul(
                    out=warm_ps[:, :], lhsT=ones128[:, :],
                    rhs=stage[:, :].bitcast(F32R),
                    start=True, stop=True, skip_group_check=True,
                )
            ones2f8 = pp.tile([128, 2, 16], F8)  # DoubleRow denom lhsT
            nc.vector.tensor_copy(ones2f8[:, :, 0], stage[:, 0:2])
            ones_col = pp.tile([1, 128], F32R)   # K=1 row-broadcast lhsT
            nc.vector.tensor_copy(ones_col[:, :], stage[0:1, :])
            epsb = pp.tile([1, 1], F32)          # LN epsilon bias
            nc.vector.memset(epsb[:, :], LN_EPS)

            # ---------------- input DMAs on 4 parallel queues --------
            wk_sb = pp.tile([128, 2, C], F8)
            wv_sb = pp.tile([128, 2, C], F8)
            wq_sb = pp.tile([128, 2, C], F8)
            wo_sb = pp.tile([128, 2, C], F8)
            pvec = pp.tile([128, 10], F32)
            lowf8_sb = pp.tile([128, 2, NQ], F8)
            low_sb = pp.tile([128, 2, NQ], F32R)
            for j in range(2):
                nc.scalar.dma_start(out=wk_sb[:, j, :], in_=wk_d[ds(j * 128, 128), :])
            nc.gpsimd.dma_start(out=pvec[:, :], in_=pvec_d[:, :])
            for j in range(2):
                nc.gpsimd.dma_start(out=wq_sb[:, j, :], in_=wq_d[ds(j * 128, 128), :])
            for j in range(2):
                nc.gpsimd.dma_start(out=wv_sb[:, j, :], in_=wv_d[ds(j * 128, 128), :])
            for j in range(2):
                nc.gpsimd.dma_start(out=wo_sb[:, j, :], in_=wo_d[ds(j * 128, 128), :])
            hi_tiles = [
                high_pool.tile([128, 2, 1024], F8, name=f"hi{r}")
                for r in range(NKR)
            ]
            # range 0 rides first, split in h-halves so the very first
            # K-projection matmul waits on a [128,512] transfer only
            for h in range(2):
                for j in range(2):
                    nc.sync.dma_start(
                        out=hi_tiles[0][:, j, ds(h * 512, 512)],
                        in_=high_d[ds(j * 128, 128), ds(h * 512, 512)],
                    )
            # block-0 slice of lowf8 first (q_proj(0) gates the exp
            # stream), then the hi ranges (k_unit(1,0) needs r1 by ~10.5us),
            # then the rest
            for j in range(2):
                nc.sync.dma_start(out=lowf8_sb[:, j, ds(0, QB)],
                                  in_=lowf8_d[ds(j * 128, 128), ds(0, QB)])
            for r in range(1, NKR):
                for j in range(2):
                    nc.sync.dma_start(
                        out=hi_tiles[r][:, j, :],
                        in_=high_d[ds(j * 128, 128), ds(r * 1024, 1024)],
                    )
            for j in range(2):
                nc.sync.dma_start(out=lowf8_sb[:, j, ds(QB, NQ - QB)],
                                  in_=lowf8_d[ds(j * 128, 128), ds(QB, NQ - QB)])
            for j in range(2):
                nc.sync.dma_start(out=low_sb[:, j, :], in_=low_d[ds(j * 128, 128), :])

            QBIAS, KBIAS, OBIAS, LNG, LNB = 0, 2, 4, 6, 8

            kt_sb = [
                pp.tile([128, 2, 1024], F8, name=f"kt{r}", tag=f"kt{r}")
                for r in range(NKR)
            ]
            v_sb = [
                pp.tile([128, 8, C], F8, name=f"v{r}", tag=f"v{r}")
                for r in range(NKR)
            ]
            qt_all = pp.tile([128, 2, NQ], F8)

            # ---------------- work units ----------------
            def k_unit(r, h, split_j=False):
                # K^T: out [cout, k] = sum_cin wk[cin, cout] high[cin, k]
                for c in range(2):
                    kps = acc_ps.tile([128, 512], F32, tag="acc")
                    if split_j:
                        # first matmuls only need the first DMA chunks
                        for j in range(2):
                            nc.tensor.matmul(
                                out=kps[:, :],
                                lhsT=wk_sb[:, j, ds(c * 128, 128)],
                                rhs=hi_tiles[r][:, j, ds(h * 512, 512)],
                                start=(j == 0), stop=(j == 1),
                            )
                    else:
                        nc.tensor.matmul(
                            out=kps[:, :],
                            lhsT=wk_sb[:, :, ds(c * 128, 128)],
                            rhs=hi_tiles[r][:, :, ds(h * 512, 512)],
                            start=True, stop=True,
                            perf_mode=DR,
                        )
                    # K bias dropped: a k-independent logit shift per query,
                    # exactly cancelled by softmax
                    nc.vector.tensor_copy(
                        kt_sb[r][:, c, ds(h * 512, 512)], kps[:, :]
                    )

            def v_unit(r, up):
                # V: out [k, cout] = sum_cin high[cin, k] wv[cin, cout]
                # DR over the cin halves; last range evacuates on ACT to
                # balance the preamble DVE load
                vps = acc_ps.tile([128, 2, C], F32, tag="acc")
                for i in range(2):
                    u = up * 2 + i
                    nc.tensor.matmul(
                        out=vps[:, i, :],
                        lhsT=hi_tiles[r][:, :, ds(u * 128, 128)],
                        rhs=wv_sb[:, :, :],
                        start=True, stop=True,
                        perf_mode=DR,
                    )
                if r >= 2:
                    nc.scalar.activation(
                        out=v_sb[r][:, ds(up * 2, 2), :], in_=vps[:, :, :],
                        func=AF.Copy,
                    )
                else:
                    nc.vector.tensor_copy(
                        v_sb[r][:, ds(up * 2, 2), :], vps[:, :, :]
                    )

            def q_proj(qb4):
                for c in range(2):
                    qps = acc_ps.tile([128, QB], F32, tag="acc")
                    nc.tensor.matmul(
                        out=qps[:, :],
                        lhsT=wq_sb[:, :, ds(c * 128, 128)],
                        rhs=lowf8_sb[:, :, ds(qb4 * QB, QB)],
                        start=True, stop=True,
                        perf_mode=DR,
                    )
                    nc.vector.tensor_scalar_add(
                        out=qt_all[:, c, ds(qb4 * QB, QB)], in0=qps[:, :],
                        scalar1=pvec[:, ds(QBIAS + c, 1)],
                    )

            def alloc_quarters(b):
                return [
                    pt_pool.tile([128, 8, QB], F8, tag="ptq", name=f"ptq{g}")
                    for g in range(4)
                ]

            quarters = {}

            def s_single(b, si):
                # 1 si = 2 key chunks: 2 S matmuls + 1 exp
                qsl = ds(b * QB, QB)
                sps = st_ps.tile([128, 2, QB], F32, tag="st")
                for u in range(2):
                    kc = si * 2 + u
                    nc.tensor.matmul(
                        out=sps[:, u, :],
                        lhsT=kt_sb[kc // 8][:, :, ds((kc % 8) * 128, 128)],
                        rhs=qt_all[:, :, qsl],
                        start=True, stop=True,
                        perf_mode=DR,
                    )
                nc.scalar.activation(
                    out=quarters[b][si // 4][:, ds((si % 4) * 2, 2), :],
                    in_=sps[:, :, :],
                    func=AF.Exp,
                    scale=EXP_SCALE,
                )

            def s_pair(b, p):
                s_single(b, 2 * p)
                s_single(b, 2 * p + 1)

            def denom_part(b, t0, t1, dps=None):
                # split accumulation: t12-15 can be emitted after other PE
                # work so the last exps of block b have time to land
                if dps is None:
                    dps = row_ps.tile([1, QB], F32, tag="row")
                for t in range(t0, t1):
                    nc.tensor.matmul(
                        out=dps[:, :],
                        lhsT=ones2f8[:, :, 0:1],
                        rhs=quarters[b][t // 4][:, ds((t % 4) * 2, 2), :],
                        start=(t == 0), stop=(t == t1 - 1),
                        perf_mode=DR,
                        skip_group_check=True,
                    )
                return dps

            def pv_part(b, c, t0, t1, ops):
                for t in range(t0, t1):
                    nc.tensor.matmul(
                        out=ops[:, :],
                        lhsT=v_sb[t // 4][:, ds((t % 4) * 2, 2),
                                         ds(c * 128, 128)],
                        rhs=quarters[b][t // 4][:, ds((t % 4) * 2, 2), :],
                        start=(t == 0), stop=(t == t1 - 1),
                        perf_mode=DR,
                        skip_group_check=True,
                    )

            def outproj_y(b, ot, rcp_rep, qo=0, ql=QB):
                qsl = ds(b * QB + qo, ql)
                y_sb = ot_pool.tile([128, 2, ql], F32R, tag="y",
                                    name=f"y{b}_{qo}")
                for c in range(2):
                    pps = acc_ps.tile([128, ql], F32, tag="acc")
                    nc.tensor.matmul(
                        out=pps[:, :],
                        lhsT=wo_sb[:, :, ds(c * 128, 128)],
                        rhs=ot[:, :, ds(qo, ql)],
                        start=True, stop=True,
                        perf_mode=DR,
                    )
                    ysc = scr_pool.tile([128, ql], F32, tag="scr")
                    nc.vector.tensor_mul(
                        out=ysc[:, :], in0=pps[:, :], in1=rcp_rep[:, ds(qo, ql)]
                    )
                    nc.vector.scalar_tensor_tensor(
                        out=y_sb[:, c, :],
                        in0=ysc[:, :],
                        scalar=pvec[:, ds(OBIAS + c, 1)],
                        in1=low_sb[:, c, qsl].bitcast(F32),
                        op0=OP.add, op1=OP.add,
                    )
                return y_sb

            def stats_ln_a(b, y_sb):
                sy_ps = row_ps.tile([1, QB], F32, tag="row")
                for c in range(2):
                    nc.tensor.matmul(
                        out=sy_ps[:, :], lhsT=ones128[:, :],
                        rhs=y_sb[:, c, :], start=(c == 0), stop=(c == 1),
                    )
                murow = row_pool.tile([1, QB], F32, tag="murow")
                nc.vector.tensor_scalar_mul(
                    out=murow[:, :], in0=sy_ps[:, :], scalar1=1.0 / C
                )
                mu_rep = scr_pool.tile([128, QB], F32, tag="murep")
                nc.gpsimd.partition_broadcast(mu_rep[:, :], murow[:, :])
                return murow, mu_rep

            def stats_ln_b(b, y_sb, murow):
                sy2_ps = row_ps.tile([1, QB], F32, tag="row")
                for c in range(2):
                    ysq = scr_pool.tile([128, QB], F32R, tag="ysq")
                    nc.vector.tensor_mul(
                        out=ysq[:, :],
                        in0=y_sb[:, c, :].bitcast(F32),
                        in1=y_sb[:, c, :].bitcast(F32),
                    )
                    nc.tensor.matmul(
                        out=sy2_ps[:, :], lhsT=ones128[:, :],
                        rhs=ysq[:, :], start=(c == 0), stop=(c == 1),
                    )
                # C*var = sy2 - C*mu^2 ; rstd = exp(-0.5 ln((C var)/C + eps))
                mu2row = row_pool.tile([1, QB], F32, tag="mu2row")
                nc.vector.tensor_mul(
                    out=mu2row[:, :], in0=murow[:, :], in1=murow[:, :],
                )
                varrow = row_pool.tile([1, QB], F32, tag="varrow")
                nc.vector.scalar_tensor_tensor(
                    out=varrow[:, :], in0=mu2row[:, :], scalar=-float(C),
                    in1=sy2_ps[:, :], op0=OP.mult, op1=OP.add,
                )
                lnv = row_pool.tile([1, QB], F32, tag="lnv")
                nc.scalar.activation(
                    out=lnv[:, :], in_=varrow[:, :], func=AF.Ln,
                    scale=1.0 / C, bias=epsb[:, :],
                )
                rstdrow = row_pool.tile([1, QB], F32, tag="rstdrow")
                nc.scalar.activation(
                    out=rstdrow[:, :], in_=lnv[:, :], func=AF.Exp, scale=-0.5
                )
                rs_rep = scr_pool.tile([128, QB], F32, tag="rsrep")
                nc.gpsimd.partition_broadcast(rs_rep[:, :], rstdrow[:, :])
                return rs_rep

            def stats_ln_c(b, y_sb, mu_rep, rs_rep):
                qsl = ds(b * QB, QB)
                for c in range(2):
                    yn = scr_pool.tile([128, QB], F32, tag="scr")
                    nc.vector.tensor_sub(
                        out=yn[:, :],
                        in0=y_sb[:, c, :].bitcast(F32),
                        in1=mu_rep[:, :],
                    )
                    nc.vector.tensor_mul(
                        out=yn[:, :], in0=yn[:, :], in1=rs_rep[:, :]
                    )
                    osb = out_pool.tile([128, QB], F32)
                    nc.vector.tensor_scalar(
                        out=osb[:, :], in0=yn[:, :],
                        scalar1=pvec[:, ds(LNG + c, 1)],
                        scalar2=pvec[:, ds(LNB + c, 1)],
                        op0=OP.mult, op1=OP.add,
                    )
                    nc.sync.dma_start(
                        out=out_d[ds(c * 128, 128), qsl], in_=osb[:, :]
                    )

            def stats_ln_last(b, ys, nh, ql):
                # span-critical tail, nh halves stage-interleaved so each
                # engine queue alternates halves and the serial chain of
                # one half hides behind the matmuls of the next.  murow on
                # ACT; rstd broadcast via a K=1 PE matmul into psum.
                sy_ps, murow, sy2_ps = {}, {}, {}
                mu2row, varrow, lnv, rstdrow = {}, {}, {}, {}
                mu_rep, rs_ps = {}, {}
                for h in range(nh):
                    sy_ps[h] = st_ps.tile([1, ql], F32, tag="st",
                                          name=f"syp{h}")
                    for c in range(2):
                        nc.tensor.matmul(
                            out=sy_ps[h][:, :], lhsT=ones128[:, :],
                            rhs=ys[h][:, c, :], start=(c == 0), stop=(c == 1),
                        )
                    murow[h] = row_pool.tile([1, ql], F32, tag="murow",
                                             name=f"mur{h}")
                    nc.scalar.activation(
                        out=murow[h][:, :], in_=sy_ps[h][:, :], func=AF.Copy,
                        scale=1.0 / C,
                    )
                    sy2_ps[h] = st_ps.tile([1, ql], F32, tag="st",
                                           name=f"sy2p{h}")
                    for c in range(2):
                        ysq = scr_pool.tile([128, ql], F32R, tag="ysq")
                        nc.vector.tensor_mul(
                            out=ysq[:, :],
                            in0=ys[h][:, c, :].bitcast(F32),
                            in1=ys[h][:, c, :].bitcast(F32),
                        )
                        nc.tensor.matmul(
                            out=sy2_ps[h][:, :], lhsT=ones128[:, :],
                            rhs=ysq[:, :], start=(c == 0), stop=(c == 1),
                        )
                for h in range(nh):
                    mu2row[h] = row_pool.tile([1, ql], F32, tag="mu2row",
                                              name=f"mu2r{h}")
                    nc.vector.tensor_mul(
                        out=mu2row[h][:, :], in0=murow[h][:, :],
                        in1=murow[h][:, :],
                    )
                    varrow[h] = row_pool.tile([1, ql], F32, tag="varrow",
                                              name=f"varr{h}")
                    nc.vector.scalar_tensor_tensor(
                        out=varrow[h][:, :], in0=mu2row[h][:, :],
                        scalar=-float(C),
                        in1=sy2_ps[h][:, :], op0=OP.mult, op1=OP.add,
                    )
                    mu_rep[h] = scr_pool.tile([128, ql], F32, tag="murep",
                                              name=f"murep{h}")
                    nc.gpsimd.partition_broadcast(mu_rep[h][:, :],
                                                  murow[h][:, :])
                for h in range(nh):
                    lnv[h] = row_pool.tile([1, ql], F32, tag="lnv",
                                           name=f"lnv{h}")
                    nc.scalar.activation(
                        out=lnv[h][:, :], in_=varrow[h][:, :], func=AF.Ln,
                        scale=1.0 / C, bias=epsb[:, :],
                    )
                    rstdrow[h] = row_pool.tile([1, ql], F32R, tag="rstdrow",
                                               name=f"rstdr{h}")
                    nc.scalar.activation(
                        out=rstdrow[h][:, :], in_=lnv[h][:, :], func=AF.Exp,
                        scale=-0.5,
                    )
                    rs_ps[h] = acc_ps.tile([128, ql], F32, tag="acc",
                                           name=f"rsps{h}")
                    nc.tensor.matmul(
                        out=rs_ps[h][:, :], lhsT=ones_col[:, :],
                        rhs=rstdrow[h][:, :], start=True, stop=True,
                    )
                for h in range(nh):
                    qsl = ds(b * QB + h * ql, ql)
                    for c in range(2):
                        yn = scr_pool.tile([128, ql], F32, tag="scr")
                        nc.vector.tensor_sub(
                            out=yn[:, :],
                            in0=ys[h][:, c, :].bitcast(F32),
                            in1=mu_rep[h][:, :],
                        )
                        nc.vector.tensor_mul(
                            out=yn[:, :], in0=yn[:, :], in1=rs_ps[h][:, :]
                        )
                        osb = out_pool.tile([128, ql], F32)
                        nc.vector.tensor_scalar(
                            out=osb[:, :], in0=yn[:, :],
                            scalar1=pvec[:, ds(LNG + c, 1)],
                            scalar2=pvec[:, ds(LNB + c, 1)],
                            op0=OP.mult, op1=OP.add,
                        )
                        nc.sync.dma_start(
                            out=out_d[ds(c * 128, 128), qsl], in_=osb[:, :]
                        )

            # ---------------- preamble: projections woven 1:1 with
            # block-0 S singles so the ACT exp stream starts ~10us in and
            # never starves, and no S matmul parks the in-order PE queue
            # (its psum buffer is 2 exps back, covered by a heavy unit)
            quarters[0] = alloc_quarters(0)
            k_unit(0, 0, split_j=True)
            q_proj(0)
            k_unit(0, 1)
            heavies = (
                [(k_unit, 1, 0), (k_unit, 1, 1), (k_unit, 2, 0),
                 (k_unit, 2, 1), (k_unit, 3, 0), (k_unit, 3, 1)]
                + [(v_unit, r, up) for r in range(3) for up in range(4)
                   ][:10]
            )
            for i in range(16):
                fn, a0, a1 = heavies[i]
                fn(a0, a1)
                s_single(0, i)
            v_unit(2, 2)
            q_proj(1)
            quarters[1] = alloc_quarters(1)
            s_single(1, 0)
            v_unit(2, 3)
            s_single(1, 1)
            v_unit(3, 0)
            s_single(1, 2)
            v_unit(3, 1)
            s_single(1, 3)
            v_unit(3, 2)
            s_single(1, 4)
            v_unit(3, 3)
            s_single(1, 5)
            q_proj(2)
            q_proj(3)
            dps = {0: denom_part(0, 0, 16)}

            # ---------------- steady state (b = 0..2) ----------------
            # block nb's S singles woven between ~1-3us chunks of block
            # b's PV/outproj/LN/denominator matmuls; block 3's PV t0-11
            # pre-woven into b=2 so the final iteration is tiny
            pv3 = {}
            sched = {
                0: [[6], [7], [8], [9], [10], [11], [12], [13]],
                1: [[0, 1], [2, 3], [4, 5], [6, 7], [8, 9], [10, 11],
                    [12], [13]],
                2: [[0, 1], [2, 3], [4, 5], [6, 7], [8, 9], [10, 11],
                    [12], [13]],
            }
            for b in range(NQB - 1):
                nb = b + 1
                # 1/denom: single custom-DVE op (ACT stays exp-only)
                rcprow = row_pool.tile([1, QB], F32, tag="rcprow",
                                       name=f"rcprow{b}")
                nc.vector.reciprocal_approx_fast(
                    out=rcprow[:, :], in_=dps[b][:, :]
                )
                rcp_rep = scr_pool.tile([128, QB], F32, tag="rcprep",
                                        name=f"rcprep{b}")
                nc.gpsimd.partition_broadcast(rcp_rep[:, :], rcprow[:, :])
                if b >= 1:
                    quarters[nb] = alloc_quarters(nb)
                Wl = sched[b]

                def weave(units, nb=nb):
                    for si in units:
                        s_single(nb, si)

                weave(Wl[0])
                ot = ot_pool.tile([128, 2, QB], F8, tag="ot", name=f"ot{b}")
                ops0 = acc_ps.tile([128, QB], F32, tag="acc")
                pv_part(b, 0, 0, 8, ops0)
                weave(Wl[1])
                pv_part(b, 0, 8, 16, ops0)
                nc.vector.tensor_copy(ot[:, 0, :], ops0[:, :])
                weave(Wl[2])
                ops1 = acc_ps.tile([128, QB], F32, tag="acc")
                pv_part(b, 1, 0, 8, ops1)
                weave(Wl[3])
                pv_part(b, 1, 8, 16, ops1)
                nc.vector.tensor_copy(ot[:, 1, :], ops1[:, :])
                weave(Wl[4])
                y_b = outproj_y(b, ot, rcp_rep)
                weave(Wl[5])
                murow, mu_rep = stats_ln_a(b, y_b)
                weave(Wl[6])
                if b == 2:
                    # pre-run block 3's PV while its exps are landing
                    pv3["ops0"] = acc_ps.tile([128, QB], F32, tag="acc",
                                              name="pv3c0")
                    pv_part(3, 0, 0, 8, pv3["ops0"])
                rs_rep = stats_ln_b(b, y_b, murow)
                weave(Wl[7])
                if b == 2:
                    pv_part(3, 0, 8, 12, pv3["ops0"])
                    pv3["ops1"] = acc_ps.tile([128, QB], F32, tag="acc",
                                              name="pv3c1")
                    pv_part(3, 1, 0, 8, pv3["ops1"])
                s_single(nb, 14)
                dps[nb] = denom_part(nb, 0, 15)
                stats_ln_c(b, y_b, mu_rep, rs_rep)
                s_single(nb, 15)
                if b == 2:
                    pv_part(3, 1, 8, 12, pv3["ops1"])
                denom_part(nb, 15, 16, dps[nb])

            # ---------------- tail: block 3 ----------------
            b = 3
            # recip on the now-idle ACT; dps[3] is complete
            lnd = row_pool.tile([1, QB], F32, tag="lnd")
            nc.scalar.activation(out=lnd[:, :], in_=dps[3][:, :], func=AF.Ln)
            rcprow3 = row_pool.tile([1, QB], F32, tag="rcprow3")
            nc.scalar.activation(out=rcprow3[:, :], in_=lnd[:, :],
                                 func=AF.Exp, scale=-1.0)
            rcp_rep3 = scr_pool.tile([128, QB], F32, tag="rcprep",
                                     name="rcprep3")
            nc.gpsimd.partition_broadcast(rcp_rep3[:, :], rcprow3[:, :])
            ot = ot_pool.tile([128, 2, QB], F8, tag="ot", name="ot3")
            pv_part(3, 0, 12, 16, pv3["ops0"])
            nc.scalar.activation(out=ot[:, 0, :], in_=pv3["ops0"][:, :],
                                 func=AF.Copy)
            pv_part(3, 1, 12, 16, pv3["ops1"])
            nc.scalar.activation(out=ot[:, 1, :], in_=pv3["ops1"][:, :],
                                 func=AF.Copy)
            # tail halves: emit both outprojs first, then the LN stages
            # interleaved, so no half's matmuls park behind the other's chain
            ys = {}
            for half in range(2):
                ys[half] = outproj_y(b, ot, rcp_rep3, qo=half * 256, ql=256)
            stats_ln_last(b, ys, nh=2, ql=256)

    # Force Exp and Ln to resolve to the one table set containing both
    # (the default chooser alternates exp_and_others <-> natural_log_exp,
    # paying a ~1.3us table load per switch, ~17 loads per kernel).
    import bass_rust as _br
    from concourse.hw_specs import get_activation_tables as _gat

    def _patched_act_loads():
        has_act = any(
            isinstance(i, mybir.InstActivation)
            for blk in nc.main_func.blocks for i in blk.instructions
        )
        if not has_act:
            return
        tables = []
        for name, fns in _gat(nc.m.arch).items():
            if name != "natural_log_exp_and_others":
                fns = fns - {AF.Exp, AF.Ln}
            tables.append((name, fns))
        _br.insert_act_table_loads(nc, tables)

    nc.insert_act_table_loads = _patched_act_loads
    nc.compile()
    return nc


def get_nc():
    if "nc" not in _CACHE:
        _CACHE["nc"] = _build_nc()
    return _CACHE["nc"]


def make_in_maps(low, high, q_w, q_b, k_w, k_b, v_w, v_b, o_w, o_b, ln_g, ln_b):
    low_r = np.asarray(low, np.float32).reshape(B, C, N)
    high_r = np.asarray(high, np.float32).reshape(B, C, N)
    f32 = lambda x: np.ascontiguousarray(np.asarray(x, np.float32))
    f8 = lambda x: np.ascontiguousarray(
        np.asarray(x, np.float32).astype(ml_dtypes.float8_e4m3)
    )
    # v-bias is exactly equivalent to an out-proj bias shift because the
    # softmax rows sum to one: attn @ (V + 1 vb^T) @ o_w^T = attn @ V @ o_w^T
    # + (o_w @ v_b)^T, so fold it on the host.
    ob_eff = np.asarray(o_b, np.float32) + np.asarray(o_w, np.float32) @ np.asarray(v_b, np.float32)
    pv_cols = []
    for v in [np.asarray(q_b, np.float32) * QK_PRE,
              np.asarray(k_b, np.float32) * QK_PRE,
              ob_eff, ln_g, ln_b]:
        pv_cols.append(np.asarray(v, np.float32).reshape(2, 128).T)
    shared = {
        "wq": f8(np.asarray(q_w, np.float32).T * QK_PRE),
        "wk": f8(np.asarray(k_w, np.float32).T * QK_PRE),
        "wv": f8(np.asarray(v_w, np.float32).T),
        "wo": f8(np.asarray(o_w, np.float32).T),
        "pvec": f32(np.concatenate(pv_cols, axis=1)),
    }
    in_maps = []
    for i in range(8):
        bidx, h = i // 2, i % 2
        lo = low_r[bidx][:, h * NQ:(h + 1) * NQ]
        in_maps.append({
            "low": f32(lo),
            "lowf8": f8(lo),
            "high": f8(high_r[bidx]),
            **shared,
        })
    return in_maps


def assemble(results):
    out = np.empty((B, C, N), np.float32)
    for i in range(8):
        bidx, h = i // 2, i % 2
        out[bidx][:, h * NQ:(h + 1) * NQ] = results[i]["out"]
    return out.reshape(B, C, 64, 64)


def kernel(**inputs) -> np.ndarray:
    nc = get_nc()
    in_maps = make_in_maps(**inputs)
    res = run_bass_kernel_spmd(nc, in_maps, core_ids=list(range(8)))
    return assemble(res.results)


if __name__ == "__main__":
    pass


# revision 19
# speedup vs baseline: 1.1948x; 1.0594x over previous
"""ContentGuidedAttention Trainium2 kernel.

Full NxN single-head cross-attention + out-proj + residual + LayerNorm,
for B=4, C=256, H=W=64 (N=4096 tokens), distributed over 8 NeuronCores:
core i handles batch i//2, query-half i%2 (2048 queries, all 4096 keys).
No collectives: K/V are computed redundantly on the two cores sharing a
batch (~5% extra FLOPs).

Layout strategy (channel-major, zero transposes, fp8 DoubleRow on every
matmul the PE streams):
  - Q^T/K^T as [C, n] fp8e4 (q/k weights prescaled by 16, compensated in
    the softmax exp scale); V token-major [n, C] fp8e4 via DR matmuls,
    evacuated on GpSimd (DVE is the preamble bottleneck)
  - S^T = K Q^T fp8 DR; exp on ACT -> P^T fp8e4
  - softmax denominator: DR ones-vector matmuls accumulate into a [1, q]
    psum row; 1/denom via a single DVE reciprocal_approx_fast (keeps the
    ACT queue exp-only), then gpsimd partition_broadcast
  - PV: O^T[c, q] = sum_k V[k,c] P^T[k,q], fp8 DR; O^T evacuated to fp8
    so the out-proj is DR too; residual uses a separate f32 copy of low
  - scheduling: the steady-state loop WEAVES 2-si S-matmul pairs between
    ~1.5us chunks of PV/outproj/LN/denominator matmuls so the in-order
    PE queue never parks behind an S matmul whose psum buffer is still
    being drained by ACT (st_ps has only 2 bufs), and ACT always has exp
    work queued.  Same fine-grained interleave in the projection
    preamble.  ~35 dummy 128-col matmuls at t~3.5us warm the PE HAM
    clock-gate before real work; inputs ride 4 DMA queues in parallel.
  - tail: last block's out-proj + LN run as two 256-query halves so the
    serial ACT/DVE chain of one half overlaps the other's matmuls.
"""

import ml_dtypes
import numpy as np

import concourse.bass as bass
import concourse.mybir as mybir
import concourse.tile as tile
from concourse import bacc
from concourse.bass import ds, ts
from concourse.bass_utils import run_bass_kernel_spmd

F32 = mybir.dt.float32
F32R = mybir.dt.float32r
BF16 = mybir.dt.bfloat16
F8 = mybir.dt.float8e4
AF = mybir.ActivationFunctionType
OP = mybir.AluOpType
DR = mybir.MatmulPerfMode.DoubleRow

B = 4
C = 256
N = 4096          # tokens per batch
NQ = 2048         # queries per core
QB = 512          # query block
NQB = NQ // QB    # 4
NKC = N // 128    # 32 key chunks
NKR = 4           # key ranges (1024 keys each) for K^T / V tiles
QK_PRE = 16.0     # host-side prescale on q/k weights (fp8 range centering)
SCALE = (C // 8) ** -0.5
EXP_SCALE = SCALE / (QK_PRE * QK_PRE)
LN_EPS = 1e-5

_CACHE = {}


def _build_nc():
    nc = bacc.Bacc("TRN2", target_bir_lowering=False, debug=False)

    low_d = nc.declare_dram_parameter("low", [C, NQ], F32R, isOutput=False)
    lowf8_d = nc.declare_dram_parameter("lowf8", [C, NQ], F8, isOutput=False)
    high_d = nc.declare_dram_parameter("high", [C, N], F8, isOutput=False)
    # weights are passed pre-transposed: [c_in, c_out], fp8
    wq_d = nc.declare_dram_parameter("wq", [C, C], F8, isOutput=False)
    wk_d = nc.declare_dram_parameter("wk", [C, C], F8, isOutput=False)
    wv_d = nc.declare_dram_parameter("wv", [C, C], F8, isOutput=False)
    wo_d = nc.declare_dram_parameter("wo", [C, C], F8, isOutput=False)
    # qb, kb, ob, lng, lnb prepacked host-side as [128, 10]
    pvec_d = nc.declare_dram_parameter("pvec", [128, 10], F32, isOutput=False)
    out_d = nc.declare_dram_parameter("out", [C, NQ], F32, isOutput=True)

    with tile.TileContext(nc) as tc:
        with (
            tc.tile_pool(name="persist", bufs=1) as pp,
            tc.tile_pool(name="high", bufs=4) as high_pool,
            tc.tile_pool(name="pt", bufs=8) as pt_pool,
            tc.tile_pool(name="ot", bufs=3) as ot_pool,
            tc.tile_pool(name="scratch", bufs=3) as scr_pool,
            tc.tile_pool(name="rowscr", bufs=1) as row_pool,
            tc.tile_pool(name="outsb", bufs=4) as out_pool,
            tc.tile_pool(name="st_ps", bufs=2, space="PSUM") as st_ps,
            tc.tile_pool(name="acc_ps", bufs=3, space="PSUM") as acc_ps,
            tc.tile_pool(name="row_ps", bufs=1, space="PSUM") as row_ps,
        ):
            # ---------------- constants + PE warm-up ----------------
            stage = pp.tile([128, 128], F32)
            nc.vector.memset(stage[:, :], 1.0)
            ones128 = pp.tile([128, 1], F32R)    # partition-reduce lhsT
            nc.vector.tensor_copy(ones128[:, :], stage[:, 0:1])
            # tiny exp: pulls the ACT table load to ~3.6us (ACT idle)
            tinyrow = pp.tile([1, 1], F32)
            nc.scalar.activation(out=tinyrow[:, :], in_=stage[0:1, 0:1],
                                 func=AF.Exp)
            # ~10 dummy matmuls bridge the PE from the start barrier to
            # the first DMA landing; the real matmul stream then keeps the
            # HAM clock-gate window busy until it unthrottles (~11us)
            warm_ps = row_ps.tile([1, 128], F32, tag="row")
            for w in range(22):
                nc.tensor.matmul(
                    out=warm_ps[:, :], lhsT=ones128[:, :],
                    rhs=stage[:, :].bitcast(F32R),
                    start=True, stop=True, skip_group_check=True,
                )
            ones2f8 = pp.tile([128, 2, 16], F8)  # DoubleRow denom lhsT
            nc.vector.tensor_copy(ones2f8[:, :, 0], stage[:, 0:2])
            ones_col = pp.tile([1, 128], F32R)   # K=1 row-broadcast lhsT
            nc.vector.tensor_copy(ones_col[:, :], stage[0:1, :])
            epsb = pp.tile([1, 1], F32)          # LN epsilon bias
            nc.vector.memset(epsb[:, :], LN_EPS)

            # ---------------- input DMAs on 4 parallel queues --------
            wk_sb = pp.tile([128, 2, C], F8)
            wv_sb = pp.tile([128, 2, C], F8)
            wq_sb = pp.tile([128, 2, C], F8)
            wo_sb = pp.tile([128, 2, C], F8)
            pvec = pp.tile([128, 10], F32)
            lowf8_sb = pp.tile([128, 2, NQ], F8)
            low_sb = pp.tile([128, 2, NQ], F32R)
            for j in range(2):
                nc.scalar.dma_start(out=wk_sb[:, j, :], in_=wk_d[ds(j * 128, 128), :])
            nc.gpsimd.dma_start(out=pvec[:, :], in_=pvec_d[:, :])
            for j in range(2):
                nc.gpsimd.dma_start(out=wq_sb[:, j, :], in_=wq_d[ds(j * 128, 128), :])
            for j in range(2):
                nc.gpsimd.dma_start(out=wv_sb[:, j, :], in_=wv_d[ds(j * 128, 128), :])
            for j in range(2):
                nc.gpsimd.dma_start(out=wo_sb[:, j, :], in_=wo_d[ds(j * 128, 128), :])
            hi_tiles = [
                high_pool.tile([128, 2, 1024], F8, name=f"hi{r}")
                for r in range(NKR)
            ]
            # range 0 rides first, split in h-halves so the very first
            # K-projection matmul waits on a [128,512] transfer only
            for h in range(2):
                for j in range(2):
                    nc.sync.dma_start(
                        out=hi_tiles[0][:, j, ds(h * 512, 512)],
                        in_=high_d[ds(j * 128, 128), ds(h * 512, 512)],
                    )
            # block-0 slice of lowf8 first (q_proj(0) gates the exp
            # stream), then the hi ranges (k_unit(1,0) needs r1 by ~10.5us),
            # then the rest
            for j in range(2):
                nc.sync.dma_start(out=lowf8_sb[:, j, ds(0, QB)],
                                  in_=lowf8_d[ds(j * 128, 128), ds(0, QB)])
            for r in range(1, NKR):
                for j in range(2):
                    nc.sync.dma_start(
                        out=hi_tiles[r][:, j, :],
                        in_=high_d[ds(j * 128, 128), ds(r * 1024, 1024)],
                    )
            for j in range(2):
                nc.sync.dma_start(out=lowf8_sb[:, j, ds(QB, NQ - QB)],
                                  in_=lowf8_d[ds(j * 128, 128), ds(QB, NQ - QB)])
            for j in range(2):
                nc.sync.dma_start(out=low_sb[:, j, :], in_=low_d[ds(j * 128, 128), :])

            QBIAS, KBIAS, OBIAS, LNG, LNB = 0, 2, 4, 6, 8

            kt_sb = [
                pp.tile([128, 2, 1024], F8, name=f"kt{r}", tag=f"kt{r}")
                for r in range(NKR)
            ]
            v_sb = [
                pp.tile([128, 8, C], F8, name=f"v{r}", tag=f"v{r}")
                for r in range(NKR)
            ]
            qt_all = pp.tile([128, 2, NQ], F8)

            # ---------------- work units ----------------
            def k_unit(r, h, split_j=False):
                # K^T: out [cout, k] = sum_cin wk[cin, cout] high[cin, k]
                for c in range(2):
                    kps = acc_ps.tile([128, 512], F32, tag="acc")
                    if split_j:
                        # first matmuls only need the first DMA chunks
                        for j in range(2):
                            nc.tensor.matmul(
                                out=kps[:, :],
                                lhsT=wk_sb[:, j, ds(c * 128, 128)],
                                rhs=hi_tiles[r][:, j, ds(h * 512, 512)],
                                start=(j == 0), stop=(j == 1),
                            )
                    else:
                        nc.tensor.matmul(
                            out=kps[:, :],
                            lhsT=wk_sb[:, :, ds(c * 128, 128)],
                            rhs=hi_tiles[r][:, :, ds(h * 512, 512)],
                            start=True, stop=True,
                            perf_mode=DR,
                        )
                    # K bias dropped: a k-independent logit shift per query,
                    # exactly cancelled by softmax
                    nc.vector.tensor_copy(
                        kt_sb[r][:, c, ds(h * 512, 512)], kps[:, :]
                    )

            def v_unit(r, up):
                # V: out [k, cout] = sum_cin high[cin, k] wv[cin, cout]
                # DR over the cin halves; last range evacuates on ACT to
                # balance the preamble DVE load
                vps = acc_ps.tile([128, 2, C], F32, tag="acc")
                for i in range(2):
                    u = up * 2 + i
                    nc.tensor.matmul(
                        out=vps[:, i, :],
                        lhsT=hi_tiles[r][:, :, ds(u * 128, 128)],
                        rhs=wv_sb[:, :, :],
                        start=True, stop=True,
                        perf_mode=DR,
                    )
                if r >= 2:
                    nc.scalar.activation(
                        out=v_sb[r][:, ds(up * 2, 2), :], in_=vps[:, :, :],
                        func=AF.Copy,
                    )
                else:
                    nc.vector.tensor_copy(
                        v_sb[r][:, ds(up * 2, 2), :], vps[:, :, :]
                    )

            def q_proj(qb4):
                for c in range(2):
                    qps = acc_ps.tile([128, QB], F32, tag="acc")
                    nc.tensor.matmul(
                        out=qps[:, :],
                        lhsT=wq_sb[:, :, ds(c * 128, 128)],
                        rhs=lowf8_sb[:, :, ds(qb4 * QB, QB)],
                        start=True, stop=True,
                        perf_mode=DR,
                    )
                    nc.vector.tensor_scalar_add(
                        out=qt_all[:, c, ds(qb4 * QB, QB)], in0=qps[:, :],
                        scalar1=pvec[:, ds(QBIAS + c, 1)],
                    )

            def alloc_quarters(b):
                return [
                    pt_pool.tile([128, 8, QB], F8, tag="ptq", name=f"ptq{g}")
                    for g in range(4)
                ]

            quarters = {}

            def s_single(b, si):
                # 1 si = 2 key chunks: 2 S matmuls + 1 exp
                qsl = ds(b * QB, QB)
                sps = st_ps.tile([128, 2, QB], F32, tag="st")
                for u in range(2):
                    kc = si * 2 + u
                    nc.tensor.matmul(
                        out=sps[:, u, :],
                        lhsT=kt_sb[kc // 8][:, :, ds((kc % 8) * 128, 128)],
                        rhs=qt_all[:, :, qsl],
                        start=True, stop=True,
                        perf_mode=DR,
                    )
                nc.scalar.activation(
                    out=quarters[b][si // 4][:, ds((si % 4) * 2, 2), :],
                    in_=sps[:, :, :],
                    func=AF.Exp,
                    scale=EXP_SCALE,
                )

            def s_pair(b, p):
                s_single(b, 2 * p)
                s_single(b, 2 * p + 1)

            def denom_part(b, t0, t1, dps=None):
                # split accumulation: t12-15 can be emitted after other PE
                # work so the last exps of block b have time to land
                if dps is None:
                    dps = row_ps.tile([1, QB], F32, tag="row")
                for t in range(t0, t1):
                    nc.tensor.matmul(
                        out=dps[:, :],
                        lhsT=ones2f8[:, :, 0:1],
                        rhs=quarters[b][t // 4][:, ds((t % 4) * 2, 2), :],
                        start=(t == 0), stop=(t == t1 - 1),
                        perf_mode=DR,
                        skip_group_check=True,
                    )
                return dps

            def pv_part(b, c, t0, t1, ops):
                for t in range(t0, t1):
                    nc.tensor.matmul(
                        out=ops[:, :],
                        lhsT=v_sb[t // 4][:, ds((t % 4) * 2, 2),
                                         ds(c * 128, 128)],
                        rhs=quarters[b][t // 4][:, ds((t % 4) * 2, 2), :],
                        start=(t == 0), stop=(t == t1 - 1),
                        perf_mode=DR,
                        skip_group_check=True,
                    )

            def outproj_y(b, ot, rcp_rep, qo=0, ql=QB):
                qsl = ds(b * QB + qo, ql)
                y_sb = ot_pool.tile([128, 2, ql], F32R, tag="y",
                                    name=f"y{b}_{qo}")
                for c in range(2):
                    pps = acc_ps.tile([128, ql], F32, tag="acc")
                    nc.tensor.matmul(
                        out=pps[:, :],
                        lhsT=wo_sb[:, :, ds(c * 128, 128)],
                        rhs=ot[:, :, ds(qo, ql)],
                        start=True, stop=True,
                        perf_mode=DR,
                    )
                    ysc = scr_pool.tile([128, ql], F32, tag="scr")
                    nc.vector.tensor_mul(
                        out=ysc[:, :], in0=pps[:, :], in1=rcp_rep[:, ds(qo, ql)]
                    )
                    nc.vector.scalar_tensor_tensor(
                        out=y_sb[:, c, :],
                        in0=ysc[:, :],
                        scalar=pvec[:, ds(OBIAS + c, 1)],
                        in1=low_sb[:, c, qsl].bitcast(F32),
                        op0=OP.add, op1=OP.add,
                    )
                return y_sb

            def stats_ln_a(b, y_sb):
                sy_ps = row_ps.tile([1, QB], F32, tag="row")
                for c in range(2):
                    nc.tensor.matmul(
                        out=sy_ps[:, :], lhsT=ones128[:, :],
                        rhs=y_sb[:, c, :], start=(c == 0), stop=(c == 1),
                    )
                murow = row_pool.tile([1, QB], F32, tag="murow")
                nc.vector.tensor_scalar_mul(
                    out=murow[:, :], in0=sy_ps[:, :], scalar1=1.0 / C
                )
                mu_rep = scr_pool.tile([128, QB], F32, tag="murep")
                nc.gpsimd.partition_broadcast(mu_rep[:, :], murow[:, :])
                return murow, mu_rep

            def stats_ln_b(b, y_sb, murow):
                sy2_ps = row_ps.tile([1, QB], F32, tag="row")
                for c in range(2):
                    ysq = scr_pool.tile([128, QB], F32R, tag="ysq")
                    nc.vector.tensor_mul(
                        out=ysq[:, :],
                        in0=y_sb[:, c, :].bitcast(F32),
                        in1=y_sb[:, c, :].bitcast(F32),
                    )
                    nc.tensor.matmul(
                        out=sy2_ps[:, :], lhsT=ones128[:, :],
                        rhs=ysq[:, :], start=(c == 0), stop=(c == 1),
                    )
                # C*var = sy2 - C*mu^2 ; rstd = exp(-0.5 ln((C var)/C + eps))
                mu2row = row_pool.tile([1, QB], F32, tag="mu2row")
                nc.vector.tensor_mul(
                    out=mu2row[:, :], in0=murow[:, :], in1=murow[:, :],
                )
                varrow = row_pool.tile([1, QB], F32, tag="varrow")
                nc.vector.scalar_tensor_tensor(
                    out=varrow[:, :], in0=mu2row[:, :], scalar=-float(C),
                    in1=sy2_ps[:, :], op0=OP.mult, op1=OP.add,
                )
                lnv = row_pool.tile([1, QB], F32, tag="lnv")
                nc.scalar.activation(
                    out=lnv[:, :], in_=varrow[:, :], func=AF.Ln,
                    scale=1.0 / C, bias=epsb[:, :],
                )
                rstdrow = row_pool.tile([1, QB], F32, tag="rstdrow")
                nc.scalar.activation(
                    out=rstdrow[:, :], in_=lnv[:, :], func=AF.Exp, scale=-0.5
                )
                rs_rep = scr_pool.tile([128, QB], F32, tag="rsrep")
                nc.gpsimd.partition_broadcast(rs_rep[:, :], rstdrow[:, :])
                return rs_rep

            def stats_ln_c(b, y_sb, mu_rep, rs_rep):
                qsl = ds(b * QB, QB)
                for c in range(2):
                    yn = scr_pool.tile([128, QB], F32, tag="scr")
                    nc.vector.tensor_sub(
                        out=yn[:, :],
                        in0=y_sb[:, c, :].bitcast(F32),
                        in1=mu_rep[:, :],
                    )
                    nc.vector.tensor_mul(
                        out=yn[:, :], in0=yn[:, :], in1=rs_rep[:, :]
                    )
                    osb = out_pool.tile([128, QB], F32)
                    nc.vector.tensor_scalar(
                        out=osb[:, :], in0=yn[:, :],
                        scalar1=pvec[:, ds(LNG + c, 1)],
                        scalar2=pvec[:, ds(LNB + c, 1)],
                        op0=OP.mult, op1=OP.add,
                    )
                    nc.sync.dma_start(
                        out=out_d[ds(c * 128, 128), qsl], in_=osb[:, :]
                    )

            def stats_ln_last(b, ys, nh, ql):
                # span-critical tail, nh halves stage-interleaved so each
                # engine queue alternates halves and the serial chain of
                # one half hides behind the matmuls of the next.  murow on
                # ACT; rstd broadcast via a K=1 PE matmul into psum.
                sy_ps, murow, sy2_ps = {}, {}, {}
                mu2row, varrow, lnv, rstdrow = {}, {}, {}, {}
                mu_rep, rs_ps = {}, {}
                for h in range(nh):
                    sy_ps[h] = st_ps.tile([1, ql], F32, tag="st",
                                          name=f"syp{h}")
                    for c in range(2):
                        nc.tensor.matmul(
                            out=sy_ps[h][:, :], lhsT=ones128[:, :],
                            rhs=ys[h][:, c, :], start=(c == 0), stop=(c == 1),
                        )
                    murow[h] = row_pool.tile([1, ql], F32R, tag="murow",
                                             name=f"mur{h}")
                    nc.scalar.activation(
                        out=murow[h][:, :], in_=sy_ps[h][:, :], func=AF.Copy,
                        scale=1.0 / C,
                    )
                    sy2_ps[h] = st_ps.tile([1, ql], F32, tag="st",
                                           name=f"sy2p{h}")
                    for c in range(2):
                        ysq = scr_pool.tile([128, ql], F32R, tag="ysq")
                        nc.vector.tensor_mul(
                            out=ysq[:, :],
                            in0=ys[h][:, c, :].bitcast(F32),
                            in1=ys[h][:, c, :].bitcast(F32),
                        )
                        nc.tensor.matmul(
                            out=sy2_ps[h][:, :], lhsT=ones128[:, :],
                            rhs=ysq[:, :], start=(c == 0), stop=(c == 1),
                        )
                for h in range(nh):
                    mu2row[h] = row_pool.tile([1, ql], F32, tag="mu2row",
                                              name=f"mu2r{h}")
                    nc.vector.tensor_mul(
                        out=mu2row[h][:, :], in0=murow[h][:, :].bitcast(F32),
                        in1=murow[h][:, :].bitcast(F32),
                    )
                    varrow[h] = row_pool.tile([1, ql], F32, tag="varrow",
                                              name=f"varr{h}")
                    nc.vector.scalar_tensor_tensor(
                        out=varrow[h][:, :], in0=mu2row[h][:, :],
                        scalar=-float(C),
                        in1=sy2_ps[h][:, :], op0=OP.mult, op1=OP.add,
                    )
                    mu_rep[h] = acc_ps.tile([128, ql], F32, tag="acc",
                                              name=f"mups{h}")
                    nc.tensor.matmul(
                        out=mu_rep[h][:, :], lhsT=ones_col[:, :],
                        rhs=murow[h][:, :],
                        start=True, stop=True,
                    )
                for h in range(nh):
                    lnv[h] = row_pool.tile([1, ql], F32, tag="lnv",
                                           name=f"lnv{h}")
                    nc.scalar.activation(
                        out=lnv[h][:, :], in_=varrow[h][:, :], func=AF.Ln,
                        scale=1.0 / C, bias=epsb[:, :],
                    )
                    rstdrow[h] = row_pool.tile([1, ql], F32R, tag="rstdrow",
                                               name=f"rstdr{h}")
                    nc.scalar.activation(
                        out=rstdrow[h][:, :], in_=lnv[h][:, :], func=AF.Exp,
                        scale=-0.5,
                    )
                    rs_ps[h] = acc_ps.tile([128, ql], F32, tag="acc",
                                           name=f"rsps{h}")
                    nc.tensor.matmul(
                        out=rs_ps[h][:, :], lhsT=ones_col[:, :],
                        rhs=rstdrow[h][:, :], start=True, stop=True,
                    )
                for h in range(nh):
                    qsl = ds(b * QB + h * ql, ql)
                    for c in range(2):
                        yn = scr_pool.tile([128, ql], F32, tag="scr")
                        nc.vector.tensor_sub(
                            out=yn[:, :],
                            in0=ys[h][:, c, :].bitcast(F32),
                            in1=mu_rep[h][:, :],
                        )
                        nc.vector.tensor_mul(
                            out=yn[:, :], in0=yn[:, :], in1=rs_ps[h][:, :]
                        )
                        osb = out_pool.tile([128, ql], F32)
                        nc.vector.tensor_scalar(
                            out=osb[:, :], in0=yn[:, :],
                            scalar1=pvec[:, ds(LNG + c, 1)],
                            scalar2=pvec[:, ds(LNB + c, 1)],
                            op0=OP.mult, op1=OP.add,
                        )
                        nc.sync.dma_start(
                            out=out_d[ds(c * 128, 128), qsl], in_=osb[:, :]
                        )

            # ---------------- preamble: each k_unit immediately feeds its
            # dependent S singles (k(r,h) -> si {4r+2h, 4r+2h+1}) so the
            # exp stream starts ~13us in; v/q units fill the PE slack
            quarters[0] = alloc_quarters(0)
            k_unit(0, 0, split_j=True)
            q_proj(0)
            s_single(0, 0)
            s_single(0, 1)
            k_unit(0, 1)
            s_single(0, 2)
            s_single(0, 3)
            k_unit(1, 0)
            s_single(0, 4)
            v_unit(0, 0)
            s_single(0, 5)
            k_unit(1, 1)
            s_single(0, 6)
            v_unit(0, 1)
            s_single(0, 7)
            k_unit(2, 0)
            s_single(0, 8)
            v_unit(0, 2)
            s_single(0, 9)
            k_unit(2, 1)
            s_single(0, 10)
            v_unit(0, 3)
            s_single(0, 11)
            k_unit(3, 0)
            s_single(0, 12)
            v_unit(1, 0)
            s_single(0, 13)
            k_unit(3, 1)
            s_single(0, 14)
            v_unit(1, 1)
            s_single(0, 15)
            v_unit(1, 2)
            v_unit(1, 3)
            v_unit(2, 0)
            v_unit(2, 1)
            v_unit(2, 2)
            v_unit(2, 3)
            q_proj(1)
            v_unit(3, 0)
            v_unit(3, 1)
            q_proj(2)
            v_unit(3, 2)
            v_unit(3, 3)
            q_proj(3)
            dps = {0: denom_part(0, 0, 16)}

            # ---------------- steady state (b = 0..2) ----------------
            # block nb's S singles woven between ~1-3us chunks of block
            # b's PV/outproj/LN/denominator matmuls; block 3's PV t0-11
            # pre-woven into b=2 so the final iteration is tiny
            pv3 = {}
            sched = {
                b: [[0, 1], [2, 3], [4, 5], [6, 7], [8, 9], [10, 11],
                    [12], [13]]
                for b in range(3)
            }
            for b in range(NQB - 1):
                nb = b + 1
                # 1/denom: single custom-DVE op (ACT stays exp-only)
                rcprow = row_pool.tile([1, QB], F32, tag="rcprow",
                                       name=f"rcprow{b}")
                nc.vector.reciprocal_approx_fast(
                    out=rcprow[:, :], in_=dps[b][:, :]
                )
                rcp_rep = scr_pool.tile([128, QB], F32, tag="rcprep",
                                        name=f"rcprep{b}")
                nc.gpsimd.partition_broadcast(rcp_rep[:, :], rcprow[:, :])
                quarters[nb] = alloc_quarters(nb)
                Wl = sched[b]

                def weave(units, nb=nb):
                    for si in units:
                        s_single(nb, si)

                weave(Wl[0])
                ot = ot_pool.tile([128, 2, QB], F8, tag="ot", name=f"ot{b}")
                ops0 = acc_ps.tile([128, QB], F32, tag="acc")
                pv_part(b, 0, 0, 8, ops0)
                weave(Wl[1])
                pv_part(b, 0, 8, 16, ops0)
                nc.vector.tensor_copy(ot[:, 0, :], ops0[:, :])
                weave(Wl[2])
                ops1 = acc_ps.tile([128, QB], F32, tag="acc")
                pv_part(b, 1, 0, 8, ops1)
                weave(Wl[3])
                pv_part(b, 1, 8, 16, ops1)
                nc.vector.tensor_copy(ot[:, 1, :], ops1[:, :])
                weave(Wl[4])
                y_b = outproj_y(b, ot, rcp_rep)
                weave(Wl[5])
                murow, mu_rep = stats_ln_a(b, y_b)
                weave(Wl[6])
                if b == 2:
                    # pre-run block 3's PV while its exps are landing
                    pv3["ops0"] = acc_ps.tile([128, QB], F32, tag="acc",
                                              name="pv3c0")
                    pv_part(3, 0, 0, 8, pv3["ops0"])
                rs_rep = stats_ln_b(b, y_b, murow)
                weave(Wl[7])
                if b == 2:
                    pv_part(3, 0, 8, 12, pv3["ops0"])
                    pv3["ops1"] = acc_ps.tile([128, QB], F32, tag="acc",
                                              name="pv3c1")
                    pv_part(3, 1, 0, 8, pv3["ops1"])
                s_single(nb, 14)
                dps[nb] = denom_part(nb, 0, 15)
                stats_ln_c(b, y_b, mu_rep, rs_rep)
                s_single(nb, 15)
                if b == 2:
                    pv_part(3, 1, 8, 12, pv3["ops1"])
                denom_part(nb, 15, 16, dps[nb])

            # ---------------- tail: block 3 ----------------
            b = 3
            # recip on the now-idle ACT; dps[3] is complete
            lnd = row_pool.tile([1, QB], F32, tag="lnd")
            nc.scalar.activation(out=lnd[:, :], in_=dps[3][:, :], func=AF.Ln)
            rcprow3 = row_pool.tile([1, QB], F32, tag="rcprow3")
            nc.scalar.activation(out=rcprow3[:, :], in_=lnd[:, :],
                                 func=AF.Exp, scale=-1.0)
            rcp_rep3 = scr_pool.tile([128, QB], F32, tag="rcprep",
                                     name="rcprep3")
            nc.gpsimd.partition_broadcast(rcp_rep3[:, :], rcprow3[:, :])
            ot = ot_pool.tile([128, 2, QB], F8, tag="ot", name="ot3")
            pv_part(3, 0, 12, 16, pv3["ops0"])  # recip rows precede evacs in ACT FIFO
            nc.scalar.activation(out=ot[:, 0, :], in_=pv3["ops0"][:, :],
                                 func=AF.Copy)
            pv_part(3, 1, 12, 16, pv3["ops1"])
            nc.scalar.activation(out=ot[:, 1, :], in_=pv3["ops1"][:, :],
                                 func=AF.Copy)
            # tail halves: emit both outprojs first, then the LN stages
            # interleaved, so no half's matmuls park behind the other's chain
            ys = {}
            for half in range(2):
                ys[half] = outproj_y(b, ot, rcp_rep3, qo=half * 256, ql=256)
            stats_ln_last(b, ys, nh=2, ql=256)

    # Force Exp and Ln to resolve to the one table set containing both
    # (the default chooser alternates exp_and_others <-> natural_log_exp,
    # paying a ~1.3us table load per switch, ~17 loads per kernel).
    import bass_rust as _br
    from concourse.hw_specs import get_activation_tables as _gat

    def _patched_act_loads():
        has_act = any(
            isinstance(i, mybir.InstActivation)
            for blk in nc.main_func.blocks for i in blk.instructions
        )
        if not has_act:
            return
        tables = []
        for name, fns in _gat(nc.m.arch).items():
            if name != "natural_log_exp_and_others":
                fns = fns - {AF.Exp, AF.Ln}
            tables.append((name, fns))
        _br.insert_act_table_loads(nc, tables)

    nc.insert_act_table_loads = _patched_act_loads
    nc.compile()
    return nc


def get_nc():
    if "nc" not in _CACHE:
        _CACHE["nc"] = _build_nc()
    return _CACHE["nc"]


def make_in_maps(low, high, q_w, q_b, k_w, k_b, v_w, v_b, o_w, o_b, ln_g, ln_b):
    low_r = np.asarray(low, np.float32).reshape(B, C, N)
    high_r = np.asarray(high, np.float32).reshape(B, C, N)
    f32 = lambda x: np.ascontiguousarray(np.asarray(x, np.float32))
    f8 = lambda x: np.ascontiguousarray(
        np.asarray(x, np.float32).astype(ml_dtypes.float8_e4m3)
    )
    # v-bias is exactly equivalent to an out-proj bias shift because the
    # softmax rows sum to one: attn @ (V + 1 vb^T) @ o_w^T = attn @ V @ o_w^T
    # + (o_w @ v_b)^T, so fold it on the host.
    ob_eff = np.asarray(o_b, np.float32) + np.asarray(o_w, np.float32) @ np.asarray(v_b, np.float32)
    pv_cols = []
    for v in [np.asarray(q_b, np.float32) * QK_PRE,
              np.asarray(k_b, np.float32) * QK_PRE,
              ob_eff, ln_g, ln_b]:
        pv_cols.append(np.asarray(v, np.float32).reshape(2, 128).T)
    shared = {
        "wq": f8(np.asarray(q_w, np.float32).T * QK_PRE),
        "wk": f8(np.asarray(k_w, np.float32).T * QK_PRE),
        "wv": f8(np.asarray(v_w, np.float32).T),
        "wo": f8(np.asarray(o_w, np.float32).T),
        "pvec": f32(np.concatenate(pv_cols, axis=1)),
    }
    in_maps = []
    for i in range(8):
        bidx, h = i // 2, i % 2
        lo = low_r[bidx][:, h * NQ:(h + 1) * NQ]
        in_maps.append({
            "low": f32(lo),
            "lowf8": f8(lo),
            "high": f8(high_r[bidx]),
            **shared,
        })
    return in_maps


def assemble(results):
    out = np.empty((B, C, N), np.float32)
    for i in range(8):
        bidx, h = i // 2, i % 2
        out[bidx][:, h * NQ:(h + 1) * NQ] = results[i]["out"]
    return out.reshape(B, C, 64, 64)


def kernel(**inputs) -> np.ndarray:
    nc = get_nc()
    in_maps = make_in_maps(**inputs)
    res = run_bass_kernel_spmd(nc, in_maps, core_ids=list(range(8)))
    return assemble(res.results)


if __name__ == "__main__":
    pass


# revision 20
# speedup vs baseline: 1.1949x; 1.0001x over previous
"""ContentGuidedAttention Trainium2 kernel.

Full NxN single-head cross-attention + out-proj + residual + LayerNorm,
for B=4, C=256, H=W=64 (N=4096 tokens), distributed over 8 NeuronCores:
core i handles batch i//2, query-half i%2 (2048 queries, all 4096 keys).
No collectives: K/V are computed redundantly on the two cores sharing a
batch (~5% extra FLOPs).

Layout strategy (channel-major, zero transposes, fp8 DoubleRow on every
matmul the PE streams):
  - Q^T/K^T as [C, n] fp8e4 (q/k weights prescaled by 16, compensated in
    the softmax exp scale); V token-major [n, C] fp8e4 via DR matmuls,
    evacuated on GpSimd (DVE is the preamble bottleneck)
  - S^T = K Q^T fp8 DR; exp on ACT -> P^T fp8e4
  - softmax denominator: DR ones-vector matmuls accumulate into a [1, q]
    psum row; 1/denom via a single DVE reciprocal_approx_fast (keeps the
    ACT queue exp-only), then gpsimd partition_broadcast
  - PV: O^T[c, q] = sum_k V[k,c] P^T[k,q], fp8 DR; O^T evacuated to fp8
    so the out-proj is DR too; residual uses a separate f32 copy of low
  - scheduling: the steady-state loop WEAVES 2-si S-matmul pairs between
    ~1.5us chunks of PV/outproj/LN/denominator matmuls so the in-order
    PE queue never parks behind an S matmul whose psum buffer is still
    being drained by ACT (st_ps has only 2 bufs), and ACT always has exp
    work queued.  Same fine-grained interleave in the projection
    preamble.  ~35 dummy 128-col matmuls at t~3.5us warm the PE HAM
    clock-gate before real work; inputs ride 4 DMA queues in parallel.
  - tail: last block's out-proj + LN run as two 256-query halves so the
    serial ACT/DVE chain of one half overlaps the other's matmuls.
"""

import ml_dtypes
import numpy as np

import concourse.bass as bass
import concourse.mybir as mybir
import concourse.tile as tile
from concourse import bacc
from concourse.bass import ds, ts
from concourse.bass_utils import run_bass_kernel_spmd

F32 = mybir.dt.float32
F32R = mybir.dt.float32r
BF16 = mybir.dt.bfloat16
F8 = mybir.dt.float8e4
AF = mybir.ActivationFunctionType
OP = mybir.AluOpType
DR = mybir.MatmulPerfMode.DoubleRow

B = 4
C = 256
N = 4096          # tokens per batch
NQ = 2048         # queries per core
QB = 512          # query block
NQB = NQ // QB    # 4
NKC = N // 128    # 32 key chunks
NKR = 4           # key ranges (1024 keys each) for K^T / V tiles
QK_PRE = 16.0     # host-side prescale on q/k weights (fp8 range centering)
SCALE = (C // 8) ** -0.5
EXP_SCALE = SCALE / (QK_PRE * QK_PRE)
LN_EPS = 1e-5

_CACHE = {}


def _build_nc():
    nc = bacc.Bacc("TRN2", target_bir_lowering=False, debug=False)

    low_d = nc.declare_dram_parameter("low", [C, NQ], F32R, isOutput=False)
    lowf8_d = nc.declare_dram_parameter("lowf8", [C, NQ], F8, isOutput=False)
    high_d = nc.declare_dram_parameter("high", [C, N], F8, isOutput=False)
    # weights are passed pre-transposed: [c_in, c_out], fp8
    wq_d = nc.declare_dram_parameter("wq", [C, C], F8, isOutput=False)
    wk_d = nc.declare_dram_parameter("wk", [C, C], F8, isOutput=False)
    wv_d = nc.declare_dram_parameter("wv", [C, C], F8, isOutput=False)
    wo_d = nc.declare_dram_parameter("wo", [C, C], F8, isOutput=False)
    # qb, kb, ob, lng, lnb prepacked host-side as [128, 10]
    pvec_d = nc.declare_dram_parameter("pvec", [128, 10], F32, isOutput=False)
    out_d = nc.declare_dram_parameter("out", [C, NQ], F32, isOutput=True)

    with tile.TileContext(nc) as tc:
        with (
            tc.tile_pool(name="persist", bufs=1) as pp,
            tc.tile_pool(name="high", bufs=4) as high_pool,
            tc.tile_pool(name="pt", bufs=8) as pt_pool,
            tc.tile_pool(name="ot", bufs=3) as ot_pool,
            tc.tile_pool(name="scratch", bufs=3) as scr_pool,
            tc.tile_pool(name="rowscr", bufs=1) as row_pool,
            tc.tile_pool(name="outsb", bufs=4) as out_pool,
            tc.tile_pool(name="st_ps", bufs=2, space="PSUM") as st_ps,
            tc.tile_pool(name="acc_ps", bufs=3, space="PSUM") as acc_ps,
            tc.tile_pool(name="row_ps", bufs=1, space="PSUM") as row_ps,
        ):
            # ---------------- constants + PE warm-up ----------------
            stage = pp.tile([128, 128], F32)
            nc.vector.memset(stage[:, :], 1.0)
            ones128 = pp.tile([128, 1], F32R)    # partition-reduce lhsT
            nc.vector.tensor_copy(ones128[:, :], stage[:, 0:1])
            # tiny exp: pulls the ACT table load to ~3.6us (ACT idle)
            tinyrow = pp.tile([1, 1], F32)
            nc.scalar.activation(out=tinyrow[:, :], in_=stage[0:1, 0:1],
                                 func=AF.Exp)
            # ~10 dummy matmuls bridge the PE from the start barrier to
            # the first DMA landing; the real matmul stream then keeps the
            # HAM clock-gate window busy until it unthrottles (~11us)
            warm_ps = row_ps.tile([1, 128], F32, tag="row")
            for w in range(22):
                nc.tensor.matmul(
                    out=warm_ps[:, :], lhsT=ones128[:, :],
                    rhs=stage[:, :].bitcast(F32R),
                    start=True, stop=True, skip_group_check=True,
                )
            ones2f8 = pp.tile([128, 2, 16], F8)  # DoubleRow denom lhsT
            nc.vector.tensor_copy(ones2f8[:, :, 0], stage[:, 0:2])
            ones_col = pp.tile([1, 128], F32R)   # K=1 row-broadcast lhsT
            nc.vector.tensor_copy(ones_col[:, :], stage[0:1, :])
            epsb = pp.tile([1, 1], F32)          # LN epsilon bias
            nc.vector.memset(epsb[:, :], LN_EPS)

            # ---------------- input DMAs on 4 parallel queues --------
            wk_sb = pp.tile([128, 2, C], F8)
            wv_sb = pp.tile([128, 2, C], F8)
            wq_sb = pp.tile([128, 2, C], F8)
            wo_sb = pp.tile([128, 2, C], F8)
            pvec = pp.tile([128, 10], F32)
            lowf8_sb = pp.tile([128, 2, NQ], F8)
            low_sb = pp.tile([128, 2, NQ], F32R)
            for j in range(2):
                nc.scalar.dma_start(out=wk_sb[:, j, :], in_=wk_d[ds(j * 128, 128), :])
            nc.gpsimd.dma_start(out=pvec[:, :], in_=pvec_d[:, :])
            for j in range(2):
                nc.gpsimd.dma_start(out=wq_sb[:, j, :], in_=wq_d[ds(j * 128, 128), :])
            for j in range(2):
                nc.gpsimd.dma_start(out=wv_sb[:, j, :], in_=wv_d[ds(j * 128, 128), :])
            for j in range(2):
                nc.gpsimd.dma_start(out=wo_sb[:, j, :], in_=wo_d[ds(j * 128, 128), :])
            hi_tiles = [
                high_pool.tile([128, 2, 1024], F8, name=f"hi{r}")
                for r in range(NKR)
            ]
            # range 0 rides first, split in h-halves so the very first
            # K-projection matmul waits on a [128,512] transfer only
            for h in range(2):
                for j in range(2):
                    nc.sync.dma_start(
                        out=hi_tiles[0][:, j, ds(h * 512, 512)],
                        in_=high_d[ds(j * 128, 128), ds(h * 512, 512)],
                    )
            # block-0 slice of lowf8 first (q_proj(0) gates the exp
            # stream), then the hi ranges (k_unit(1,0) needs r1 by ~10.5us),
            # then the rest
            for j in range(2):
                nc.sync.dma_start(out=lowf8_sb[:, j, ds(0, QB)],
                                  in_=lowf8_d[ds(j * 128, 128), ds(0, QB)])
            for r in range(1, NKR):
                for j in range(2):
                    nc.sync.dma_start(
                        out=hi_tiles[r][:, j, :],
                        in_=high_d[ds(j * 128, 128), ds(r * 1024, 1024)],
                    )
            for j in range(2):
                nc.sync.dma_start(out=lowf8_sb[:, j, ds(QB, NQ - QB)],
                                  in_=lowf8_d[ds(j * 128, 128), ds(QB, NQ - QB)])
            for j in range(2):
                nc.sync.dma_start(out=low_sb[:, j, :], in_=low_d[ds(j * 128, 128), :])

            QBIAS, KBIAS, OBIAS, LNG, LNB = 0, 2, 4, 6, 8

            kt_sb = [
                pp.tile([128, 2, 1024], F8, name=f"kt{r}", tag=f"kt{r}")
                for r in range(NKR)
            ]
            v_sb = [
                pp.tile([128, 8, C], F8, name=f"v{r}", tag=f"v{r}")
                for r in range(NKR)
            ]
            qt_all = pp.tile([128, 2, NQ], F8)

            # ---------------- work units ----------------
            def k_unit(r, h, split_j=False):
                # K^T: out [cout, k] = sum_cin wk[cin, cout] high[cin, k]
                for c in range(2):
                    kps = acc_ps.tile([128, 512], F32, tag="acc")
                    if split_j:
                        # first matmuls only need the first DMA chunks
                        for j in range(2):
                            nc.tensor.matmul(
                                out=kps[:, :],
                                lhsT=wk_sb[:, j, ds(c * 128, 128)],
                                rhs=hi_tiles[r][:, j, ds(h * 512, 512)],
                                start=(j == 0), stop=(j == 1),
                            )
                    else:
                        nc.tensor.matmul(
                            out=kps[:, :],
                            lhsT=wk_sb[:, :, ds(c * 128, 128)],
                            rhs=hi_tiles[r][:, :, ds(h * 512, 512)],
                            start=True, stop=True,
                            perf_mode=DR,
                        )
                    # K bias dropped: a k-independent logit shift per query,
                    # exactly cancelled by softmax
                    nc.vector.tensor_copy(
                        kt_sb[r][:, c, ds(h * 512, 512)], kps[:, :]
                    )

            def v_unit(r, up):
                # V: out [k, cout] = sum_cin high[cin, k] wv[cin, cout]
                # DR over the cin halves; last range evacuates on ACT to
                # balance the preamble DVE load
                vps = acc_ps.tile([128, 2, C], F32, tag="acc")
                for i in range(2):
                    u = up * 2 + i
                    nc.tensor.matmul(
                        out=vps[:, i, :],
                        lhsT=hi_tiles[r][:, :, ds(u * 128, 128)],
                        rhs=wv_sb[:, :, :],
                        start=True, stop=True,
                        perf_mode=DR,
                    )
                if r >= 2:
                    nc.scalar.activation(
                        out=v_sb[r][:, ds(up * 2, 2), :], in_=vps[:, :, :],
                        func=AF.Copy,
                    )
                else:
                    nc.vector.tensor_copy(
                        v_sb[r][:, ds(up * 2, 2), :], vps[:, :, :]
                    )

            def q_proj(qb4):
                for c in range(2):
                    qps = acc_ps.tile([128, QB], F32, tag="acc")
                    nc.tensor.matmul(
                        out=qps[:, :],
                        lhsT=wq_sb[:, :, ds(c * 128, 128)],
                        rhs=lowf8_sb[:, :, ds(qb4 * QB, QB)],
                        start=True, stop=True,
                        perf_mode=DR,
                    )
                    nc.vector.tensor_scalar_add(
                        out=qt_all[:, c, ds(qb4 * QB, QB)], in0=qps[:, :],
                        scalar1=pvec[:, ds(QBIAS + c, 1)],
                    )

            def alloc_quarters(b):
                return [
                    pt_pool.tile([128, 8, QB], F8, tag="ptq", name=f"ptq{g}")
                    for g in range(4)
                ]

            quarters = {}

            def s_single(b, si):
                # 1 si = 2 key chunks: 2 S matmuls + 1 exp
                qsl = ds(b * QB, QB)
                sps = st_ps.tile([128, 2, QB], F32, tag="st")
                for u in range(2):
                    kc = si * 2 + u
                    nc.tensor.matmul(
                        out=sps[:, u, :],
                        lhsT=kt_sb[kc // 8][:, :, ds((kc % 8) * 128, 128)],
                        rhs=qt_all[:, :, qsl],
                        start=True, stop=True,
                        perf_mode=DR,
                    )
                nc.scalar.activation(
                    out=quarters[b][si // 4][:, ds((si % 4) * 2, 2), :],
                    in_=sps[:, :, :],
                    func=AF.Exp,
                    scale=EXP_SCALE,
                )

            def s_pair(b, p):
                s_single(b, 2 * p)
                s_single(b, 2 * p + 1)

            def denom_part(b, t0, t1, dps=None):
                # split accumulation: t12-15 can be emitted after other PE
                # work so the last exps of block b have time to land
                if dps is None:
                    dps = row_ps.tile([1, QB], F32, tag="row")
                for t in range(t0, t1):
                    nc.tensor.matmul(
                        out=dps[:, :],
                        lhsT=ones2f8[:, :, 0:1],
                        rhs=quarters[b][t // 4][:, ds((t % 4) * 2, 2), :],
                        start=(t == 0), stop=(t == t1 - 1),
                        perf_mode=DR,
                        skip_group_check=True,
                    )
                return dps

            def pv_part(b, c, t0, t1, ops):
                for t in range(t0, t1):
                    nc.tensor.matmul(
                        out=ops[:, :],
                        lhsT=v_sb[t // 4][:, ds((t % 4) * 2, 2),
                                         ds(c * 128, 128)],
                        rhs=quarters[b][t // 4][:, ds((t % 4) * 2, 2), :],
                        start=(t == 0), stop=(t == t1 - 1),
                        perf_mode=DR,
                        skip_group_check=True,
                    )

            def outproj_y(b, ot, rcp_rep, qo=0, ql=QB):
                qsl = ds(b * QB + qo, ql)
                y_sb = ot_pool.tile([128, 2, ql], F32R, tag="y",
                                    name=f"y{b}_{qo}")
                for c in range(2):
                    pps = acc_ps.tile([128, ql], F32, tag="acc")
                    nc.tensor.matmul(
                        out=pps[:, :],
                        lhsT=wo_sb[:, :, ds(c * 128, 128)],
                        rhs=ot[:, :, ds(qo, ql)],
                        start=True, stop=True,
                        perf_mode=DR,
                    )
                    ysc = scr_pool.tile([128, ql], F32, tag="scr")
                    nc.vector.tensor_mul(
                        out=ysc[:, :], in0=pps[:, :], in1=rcp_rep[:, ds(qo, ql)]
                    )
                    nc.vector.scalar_tensor_tensor(
                        out=y_sb[:, c, :],
                        in0=ysc[:, :],
                        scalar=pvec[:, ds(OBIAS + c, 1)],
                        in1=low_sb[:, c, qsl].bitcast(F32),
                        op0=OP.add, op1=OP.add,
                    )
                return y_sb

            def stats_ln_a(b, y_sb):
                sy_ps = row_ps.tile([1, QB], F32, tag="row")
                for c in range(2):
                    nc.tensor.matmul(
                        out=sy_ps[:, :], lhsT=ones128[:, :],
                        rhs=y_sb[:, c, :], start=(c == 0), stop=(c == 1),
                    )
                murow = row_pool.tile([1, QB], F32, tag="murow")
                nc.vector.tensor_scalar_mul(
                    out=murow[:, :], in0=sy_ps[:, :], scalar1=1.0 / C
                )
                mu_rep = scr_pool.tile([128, QB], F32, tag="murep")
                nc.gpsimd.partition_broadcast(mu_rep[:, :], murow[:, :])
                return murow, mu_rep

            def stats_ln_b(b, y_sb, murow):
                sy2_ps = row_ps.tile([1, QB], F32, tag="row")
                for c in range(2):
                    ysq = scr_pool.tile([128, QB], F32R, tag="ysq")
                    nc.vector.tensor_mul(
                        out=ysq[:, :],
                        in0=y_sb[:, c, :].bitcast(F32),
                        in1=y_sb[:, c, :].bitcast(F32),
                    )
                    nc.tensor.matmul(
                        out=sy2_ps[:, :], lhsT=ones128[:, :],
                        rhs=ysq[:, :], start=(c == 0), stop=(c == 1),
                    )
                # C*var = sy2 - C*mu^2 ; rstd = exp(-0.5 ln((C var)/C + eps))
                mu2row = row_pool.tile([1, QB], F32, tag="mu2row")
                nc.vector.tensor_mul(
                    out=mu2row[:, :], in0=murow[:, :], in1=murow[:, :],
                )
                varrow = row_pool.tile([1, QB], F32, tag="varrow")
                nc.vector.scalar_tensor_tensor(
                    out=varrow[:, :], in0=mu2row[:, :], scalar=-float(C),
                    in1=sy2_ps[:, :], op0=OP.mult, op1=OP.add,
                )
                lnv = row_pool.tile([1, QB], F32, tag="lnv")
                nc.scalar.activation(
                    out=lnv[:, :], in_=varrow[:, :], func=AF.Ln,
                    scale=1.0 / C, bias=epsb[:, :],
                )
                rstdrow = row_pool.tile([1, QB], F32, tag="rstdrow")
                nc.scalar.activation(
                    out=rstdrow[:, :], in_=lnv[:, :], func=AF.Exp, scale=-0.5
                )
                rs_rep = scr_pool.tile([128, QB], F32, tag="rsrep")
                nc.gpsimd.partition_broadcast(rs_rep[:, :], rstdrow[:, :])
                return rs_rep

            def stats_ln_c(b, y_sb, mu_rep, rs_rep):
                qsl = ds(b * QB, QB)
                for c in range(2):
                    yn = scr_pool.tile([128, QB], F32, tag="scr")
                    nc.vector.tensor_sub(
                        out=yn[:, :],
                        in0=y_sb[:, c, :].bitcast(F32),
                        in1=mu_rep[:, :],
                    )
                    nc.vector.tensor_mul(
                        out=yn[:, :], in0=yn[:, :], in1=rs_rep[:, :]
                    )
                    osb = out_pool.tile([128, QB], F32)
                    nc.vector.tensor_scalar(
                        out=osb[:, :], in0=yn[:, :],
                        scalar1=pvec[:, ds(LNG + c, 1)],
                        scalar2=pvec[:, ds(LNB + c, 1)],
                        op0=OP.mult, op1=OP.add,
                    )
                    nc.sync.dma_start(
                        out=out_d[ds(c * 128, 128), qsl], in_=osb[:, :]
                    )

            def stats_ln_last(b, ys, nh, ql):
                # span-critical tail, nh halves stage-interleaved so each
                # engine queue alternates halves and the serial chain of
                # one half hides behind the matmuls of the next.  murow on
                # ACT; rstd broadcast via a K=1 PE matmul into psum.
                sy_ps, murow, sy2_ps = {}, {}, {}
                mu2row, varrow, lnv, rstdrow = {}, {}, {}, {}
                mu_rep, rs_ps = {}, {}
                for h in range(nh):
                    sy_ps[h] = st_ps.tile([1, ql], F32, tag="st",
                                          name=f"syp{h}")
                    for c in range(2):
                        nc.tensor.matmul(
                            out=sy_ps[h][:, :], lhsT=ones128[:, :],
                            rhs=ys[h][:, c, :], start=(c == 0), stop=(c == 1),
                        )
                    murow[h] = row_pool.tile([1, ql], F32R, tag="murow",
                                             name=f"mur{h}")
                    nc.scalar.activation(
                        out=murow[h][:, :], in_=sy_ps[h][:, :], func=AF.Copy,
                        scale=1.0 / C,
                    )
                    sy2_ps[h] = st_ps.tile([1, ql], F32, tag="st",
                                           name=f"sy2p{h}")
                    for c in range(2):
                        ysq = scr_pool.tile([128, ql], F32R, tag="ysq")
                        nc.vector.tensor_mul(
                            out=ysq[:, :],
                            in0=ys[h][:, c, :].bitcast(F32),
                            in1=ys[h][:, c, :].bitcast(F32),
                        )
                        nc.tensor.matmul(
                            out=sy2_ps[h][:, :], lhsT=ones128[:, :],
                            rhs=ysq[:, :], start=(c == 0), stop=(c == 1),
                        )
                for h in range(nh):
                    mu2row[h] = row_pool.tile([1, ql], F32, tag="mu2row",
                                              name=f"mu2r{h}")
                    nc.vector.tensor_mul(
                        out=mu2row[h][:, :], in0=murow[h][:, :].bitcast(F32),
                        in1=murow[h][:, :].bitcast(F32),
                    )
                    varrow[h] = row_pool.tile([1, ql], F32, tag="varrow",
                                              name=f"varr{h}")
                    nc.vector.scalar_tensor_tensor(
                        out=varrow[h][:, :], in0=mu2row[h][:, :],
                        scalar=-float(C),
                        in1=sy2_ps[h][:, :], op0=OP.mult, op1=OP.add,
                    )
                    mu_rep[h] = acc_ps.tile([128, ql], F32, tag="acc",
                                              name=f"mups{h}")
                    nc.tensor.matmul(
                        out=mu_rep[h][:, :], lhsT=ones_col[:, :],
                        rhs=murow[h][:, :],
                        start=True, stop=True,
                    )
                for h in range(nh):
                    lnv[h] = row_pool.tile([1, ql], F32, tag="lnv",
                                           name=f"lnv{h}")
                    nc.scalar.activation(
                        out=lnv[h][:, :], in_=varrow[h][:, :], func=AF.Ln,
                        scale=1.0 / C, bias=epsb[:, :],
                    )
                    rstdrow[h] = row_pool.tile([1, ql], F32R, tag="rstdrow",
                                               name=f"rstdr{h}")
                    nc.scalar.activation(
                        out=rstdrow[h][:, :], in_=lnv[h][:, :], func=AF.Exp,
                        scale=-0.5,
                    )
                    rs_ps[h] = acc_ps.tile([128, ql], F32, tag="acc",
                                           name=f"rsps{h}")
                    nc.tensor.matmul(
                        out=rs_ps[h][:, :], lhsT=ones_col[:, :],
                        rhs=rstdrow[h][:, :], start=True, stop=True,
                    )
                for h in range(nh):
                    qsl = ds(b * QB + h * ql, ql)
                    for c in range(2):
                        yn = scr_pool.tile([128, ql], F32, tag="scr")
                        nc.vector.tensor_sub(
                            out=yn[:, :],
                            in0=ys[h][:, c, :].bitcast(F32),
                            in1=mu_rep[h][:, :],
                        )
                        nc.vector.tensor_mul(
                            out=yn[:, :], in0=yn[:, :], in1=rs_ps[h][:, :]
                        )
                        osb = out_pool.tile([128, ql], F32)
                        nc.vector.tensor_scalar(
                            out=osb[:, :], in0=yn[:, :],
                            scalar1=pvec[:, ds(LNG + c, 1)],
                            scalar2=pvec[:, ds(LNB + c, 1)],
                            op0=OP.mult, op1=OP.add,
                        )
                        eng = nc.sync if c == 0 else nc.scalar
                        eng.dma_start(
                            out=out_d[ds(c * 128, 128), qsl], in_=osb[:, :]
                        )

            # ---------------- preamble: each k_unit immediately feeds its
            # dependent S singles (k(r,h) -> si {4r+2h, 4r+2h+1}) so the
            # exp stream starts ~13us in; v/q units fill the PE slack
            quarters[0] = alloc_quarters(0)
            k_unit(0, 0, split_j=True)
            q_proj(0)
            s_single(0, 0)
            s_single(0, 1)
            k_unit(0, 1)
            s_single(0, 2)
            s_single(0, 3)
            k_unit(1, 0)
            s_single(0, 4)
            v_unit(0, 0)
            s_single(0, 5)
            k_unit(1, 1)
            s_single(0, 6)
            v_unit(0, 1)
            s_single(0, 7)
            k_unit(2, 0)
            s_single(0, 8)
            v_unit(0, 2)
            s_single(0, 9)
            k_unit(2, 1)
            s_single(0, 10)
            v_unit(0, 3)
            s_single(0, 11)
            k_unit(3, 0)
            s_single(0, 12)
            v_unit(1, 0)
            s_single(0, 13)
            k_unit(3, 1)
            s_single(0, 14)
            v_unit(1, 1)
            s_single(0, 15)
            v_unit(1, 2)
            v_unit(1, 3)
            v_unit(2, 0)
            v_unit(2, 1)
            v_unit(2, 2)
            v_unit(2, 3)
            q_proj(1)
            v_unit(3, 0)
            v_unit(3, 1)
            q_proj(2)
            v_unit(3, 2)
            v_unit(3, 3)
            q_proj(3)
            dps = {0: denom_part(0, 0, 16)}

            # ---------------- steady state (b = 0..2) ----------------
            # block nb's S singles woven between ~1-3us chunks of block
            # b's PV/outproj/LN/denominator matmuls; block 3's PV t0-11
            # pre-woven into b=2 so the final iteration is tiny
            pv3 = {}
            sched = {
                b: [[0, 1], [2, 3], [4, 5], [6, 7], [8, 9], [10, 11],
                    [12], [13]]
                for b in range(3)
            }
            sched[2] = [[0, 1], [2, 3], [4, 5], [6, 7], [8, 9], [10, 11],
                        [12, 13], [14, 15]]
            for b in range(NQB - 1):
                nb = b + 1
                # 1/denom: single custom-DVE op (ACT stays exp-only)
                rcprow = row_pool.tile([1, QB], F32, tag="rcprow",
                                       name=f"rcprow{b}")
                nc.vector.reciprocal_approx_fast(
                    out=rcprow[:, :], in_=dps[b][:, :]
                )
                rcp_rep = scr_pool.tile([128, QB], F32, tag="rcprep",
                                        name=f"rcprep{b}")
                nc.gpsimd.partition_broadcast(rcp_rep[:, :], rcprow[:, :])
                quarters[nb] = alloc_quarters(nb)
                Wl = sched[b]

                def weave(units, nb=nb):
                    for si in units:
                        s_single(nb, si)

                weave(Wl[0])
                ot = ot_pool.tile([128, 2, QB], F8, tag="ot", name=f"ot{b}")
                ops0 = acc_ps.tile([128, QB], F32, tag="acc")
                pv_part(b, 0, 0, 8, ops0)
                weave(Wl[1])
                pv_part(b, 0, 8, 16, ops0)
                nc.vector.tensor_copy(ot[:, 0, :], ops0[:, :])
                weave(Wl[2])
                ops1 = acc_ps.tile([128, QB], F32, tag="acc")
                pv_part(b, 1, 0, 8, ops1)
                weave(Wl[3])
                pv_part(b, 1, 8, 16, ops1)
                nc.vector.tensor_copy(ot[:, 1, :], ops1[:, :])
                weave(Wl[4])
                y_b = outproj_y(b, ot, rcp_rep)
                weave(Wl[5])
                murow, mu_rep = stats_ln_a(b, y_b)
                weave(Wl[6])
                if b == 2:
                    # pre-run block 3's PV while its exps are landing
                    pv3["ops0"] = acc_ps.tile([128, QB], F32, tag="acc",
                                              name="pv3c0")
                    pv_part(3, 0, 0, 8, pv3["ops0"])
                rs_rep = stats_ln_b(b, y_b, murow)
                weave(Wl[7])
                if b == 2:
                    pv_part(3, 0, 8, 12, pv3["ops0"])
                    pv3["ops1"] = acc_ps.tile([128, QB], F32, tag="acc",
                                              name="pv3c1")
                    pv_part(3, 1, 0, 8, pv3["ops1"])
                if b < 2:
                    s_single(nb, 14)
                dps[nb] = denom_part(nb, 0, 15)
                stats_ln_c(b, y_b, mu_rep, rs_rep)
                if b < 2:
                    s_single(nb, 15)
                else:
                    pv_part(3, 1, 8, 12, pv3["ops1"])
                denom_part(nb, 15, 16, dps[nb])

            # ---------------- tail: block 3 ----------------
            b = 3
            # recip on the now-idle ACT; dps[3] is complete
            lnd = row_pool.tile([1, QB], F32, tag="lnd")
            nc.scalar.activation(out=lnd[:, :], in_=dps[3][:, :], func=AF.Ln)
            rcprow3 = row_pool.tile([1, QB], F32, tag="rcprow3")
            nc.scalar.activation(out=rcprow3[:, :], in_=lnd[:, :],
                                 func=AF.Exp, scale=-1.0)
            rcp_rep3 = scr_pool.tile([128, QB], F32, tag="rcprep",
                                     name="rcprep3")
            nc.gpsimd.partition_broadcast(rcp_rep3[:, :], rcprow3[:, :])
            ot = ot_pool.tile([128, 2, QB], F8, tag="ot", name="ot3")
            pv_part(3, 0, 12, 16, pv3["ops0"])  # recip rows precede evacs in ACT FIFO
            nc.scalar.activation(out=ot[:, 0, :], in_=pv3["ops0"][:, :],
                                 func=AF.Copy)
            pv_part(3, 1, 12, 16, pv3["ops1"])
            nc.scalar.activation(out=ot[:, 1, :], in_=pv3["ops1"][:, :],
                                 func=AF.Copy)
            # tail halves: emit both outprojs first, then the LN stages
            # interleaved, so no half's matmuls park behind the other's chain
            ys = {}
            for half in range(2):
                ys[half] = outproj_y(b, ot, rcp_rep3, qo=half * 256, ql=256)
            stats_ln_last(b, ys, nh=2, ql=256)

    # Force Exp and Ln to resolve to the one table set containing both
    # (the default chooser alternates exp_and_others <-> natural_log_exp,
    # paying a ~1.3us table load per switch, ~17 loads per kernel).
    import bass_rust as _br
    from concourse.hw_specs import get_activation_tables as _gat

    def _patched_act_loads():
        has_act = any(
            isinstance(i, mybir.InstActivation)
            for blk in nc.main_func.blocks for i in blk.instructions
        )
        if not has_act:
            return
        tables = []
        for name, fns in _gat(nc.m.arch).items():
            if name != "natural_log_exp_and_others":
                fns = fns - {AF.Exp, AF.Ln}
            tables.append((name, fns))
        _br.insert_act_table_loads(nc, tables)

    nc.insert_act_table_loads = _patched_act_loads
    nc.compile()
    return nc


def get_nc():
    if "nc" not in _CACHE:
        _CACHE["nc"] = _build_nc()
    return _CACHE["nc"]


def make_in_maps(low, high, q_w, q_b, k_w, k_b, v_w, v_b, o_w, o_b, ln_g, ln_b):
    low_r = np.asarray(low, np.float32).reshape(B, C, N)
    high_r = np.asarray(high, np.float32).reshape(B, C, N)
    f32 = lambda x: np.ascontiguousarray(np.asarray(x, np.float32))
    f8 = lambda x: np.ascontiguousarray(
        np.asarray(x, np.float32).astype(ml_dtypes.float8_e4m3)
    )
    # v-bias is exactly equivalent to an out-proj bias shift because the
    # softmax rows sum to one: attn @ (V + 1 vb^T) @ o_w^T = attn @ V @ o_w^T
    # + (o_w @ v_b)^T, so fold it on the host.
    ob_eff = np.asarray(o_b, np.float32) + np.asarray(o_w, np.float32) @ np.asarray(v_b, np.float32)
    pv_cols = []
    for v in [np.asarray(q_b, np.float32) * QK_PRE,
              np.asarray(k_b, np.float32) * QK_PRE,
              ob_eff, ln_g, ln_b]:
        pv_cols.append(np.asarray(v, np.float32).reshape(2, 128).T)
    shared = {
        "wq": f8(np.asarray(q_w, np.float32).T * QK_PRE),
        "wk": f8(np.asarray(k_w, np.float32).T * QK_PRE),
        "wv": f8(np.asarray(v_w, np.float32).T),
        "wo": f8(np.asarray(o_w, np.float32).T),
        "pvec": f32(np.concatenate(pv_cols, axis=1)),
    }
    in_maps = []
    for i in range(8):
        bidx, h = i // 2, i % 2
        lo = low_r[bidx][:, h * NQ:(h + 1) * NQ]
        in_maps.append({
            "low": f32(lo),
            "lowf8": f8(lo),
            "high": f8(high_r[bidx]),
            **shared,
        })
    return in_maps


def assemble(results):
    out = np.empty((B, C, N), np.float32)
    for i in range(8):
        bidx, h = i // 2, i % 2
        out[bidx][:, h * NQ:(h + 1) * NQ] = results[i]["out"]
    return out.reshape(B, C, 64, 64)


def kernel(**inputs) -> np.ndarray:
    nc = get_nc()
    in_maps = make_in_maps(**inputs)
    res = run_bass_kernel_spmd(nc, in_maps, core_ids=list(range(8)))
    return assemble(res.results)


if __name__ == "__main__":
    pass
